# revision 26
# baseline (speedup 1.0000x reference)
"""Trainium2 Bass kernel for nn_DGCN (gnn_message_passing).

Sharding: 8 shards = (batch b in 0..3, row-half h in 0..1). Each core gets
the full 2048-node K-side tensors of its batch with the node axis ROTATED
by h*1024 so the adjacency diagonal lands at the same tile position on
every core (uniform SPMD program); the core computes rows 0..1023 of the
rotated order, which are rows [h*1024, (h+1)*1024) of the original order.

v6 — latency-first restructure of v4:
 - All per-node LN statistics land directly in chunk-column layout via
   per-chunk transposed stat matmuls (lhsT = data chunk, rhs = selector),
   eliminating every SBUF->SBUF scatter DMA round trip; only single-row
   gathers remain (split across the sync/scalar/gpsimd queues), fed by an
   XBAR DMA-transpose of the packed [128,32] stat tile (no PSUM use, so
   the land cannot deadlock against the fully-committed phase-I banks).
 - leaky_relu fuses into the PSUM->SBUF evacuation via the scalar
   engine's Prelu activation (alpha=0.01; Prelu, unlike Lrelu, lives in
   21 act-table sets so it never forces a table switch). Scalar table
   switches are monotone: sigmoid-set ops (fc, GRU) precede the single
   switch to the sqrt set, each prefetched by a dummy op.
 - The q/k projections, the K-side/x3 Gram chunk projections (krq/xrq
   off HgQ/x2a — independent of the kq evacuations), and the Gram
   accumulations are interleaved on the PE so the whole mid-section is
   tensor-throughput-bound instead of chained; evacuations alternate
   scalar Prelu / vector copy+leaky. s1/t1 row-stat matmuls issue as
   soon as ks lands; s2/t2 follow the Gram. PSUM: 5 rotating front banks
   + 3 pinned Gram accumulator banks.
 - The GCN/diag tail is pipelined per 2-chunk pair inside phase I in 3
   stages (diag/rowsum+broadcast -> GCN-2/3 -> final LN + store), each
   emitted one chunk apart so tail PSUM allocs never overlap the 8
   committed zpt banks; after the final N^2 chunk only ~one pair's tail
   remains. The last pair uses a PE transpose (PSUM is free by then).
 - Phase I (the single fused N^2 pass: q.k2 + x3-Gram bracket, relu,
   rowsum via accum_out, diag via affine_select) is unchanged
   mathematically, software-pipelined two chunks deep across all 8 PSUM
   banks.
 - Known non-fix: the PE duty-cycles between 1.2 and 2.4 GHz run-to-run
   (device power state); wall time is bimodal ~142us / ~17xus across
   runs with identical instruction streams.
"""

import sys

if '/opt/trn_rl_repo' not in sys.path:
    sys.path.insert(0, '/opt/trn_rl_repo')

from contextlib import ExitStack

import numpy as np
import ml_dtypes

import concourse.bass as bass
import concourse.tile as tile
from concourse import bacc, mybir
from concourse.bass_interp import get_hw_module
from concourse.bass_utils import run_bass_kernel_spmd

FP = mybir.dt.float32
BF = mybir.dt.bfloat16
AL = mybir.AluOpType
AF = mybir.ActivationFunctionType
AX = mybir.AxisListType

B, N, E, G, H = 4, 2048, 64, 64, 4
D = H * G          # 256
HALF = N // 2      # own rows per core
NCH = N // 128     # 16 chunks over all nodes
HCH = HALF // 128  # 8 own chunks
MB = 512
NMB = N // MB      # 4
EPS = 1e-5

# wpack (bf16 [128, WPACK_W]) column layout
W_IDB, W_WZ, W_WR, W_WH = 0, 128, 192, 256
W_QA, W_KA = 320, 576
W_FC1, W_FC2, W_FC3A = 832, 848, 850
W_W1A, W_W2A, W_W3A = 914, 978, 1042
W_SEL, W_ONE = 1106, 1108
WPACK_W = 1280
# fpack (fp32 [128, FPACK_W]) column layout
F_B, F_EPS, F_XG, F_XB3, F_BN = 0, 3, 4, 5, 8
FPACK_W = 264

_CACHE = {}


def _tp(nc, out_ap, in_ap, ident):
    k = in_ap.partition_size()
    nc.tensor.transpose(out_ap, in_ap, ident[0:k, 0:k])


def _emit(ctx: ExitStack, tc: tile.TileContext, io: dict):
    nc = tc.nc

    persist = ctx.enter_context(tc.tile_pool(name="persist", bufs=1))
    small = ctx.enter_context(tc.tile_pool(name="small", bufs=1))

    # ---------------- packed params (2 DMAs) ----------------
    wp = persist.tile([128, WPACK_W], BF, tag="wp")
    nc.sync.dma_start(wp[:], io["wpack"][:])
    fp_ = persist.tile([128, FPACK_W], FP, tag="fp_")
    nc.scalar.dma_start(fp_[:], io["fpack"][:])

    identb = wp[:, W_IDB:W_IDB + 128]
    wz = wp[:, W_WZ:W_WZ + 64]
    wr = wp[:, W_WR:W_WR + 64]
    wh = wp[:, W_WH:W_WH + 64]
    fc1s = wp[0:64, W_FC1:W_FC1 + 16]
    fc2s = wp[0:16, W_FC2:W_FC2 + 2]
    fc3s = wp[0:2, W_FC3A:W_FC3A + 64]
    fc3a = wp[0:3, W_FC3A:W_FC3A + 64]
    kA = wp[0:66, W_KA:W_KA + 256]
    w1a = wp[0:65, W_W1A:W_W1A + 64]
    w2a = wp[0:65, W_W2A:W_W2A + 64]
    w3a = wp[0:65, W_W3A:W_W3A + 64]
    sel2 = wp[:, W_SEL:W_SEL + 2]
    ones128c = wp[:, W_ONE:W_ONE + 1]
    ones64c = wp[0:64, W_ONE:W_ONE + 1]
    onesr128 = wp[0:1, W_ONE:W_ONE + 128]
    onesr64 = wp[0:1, W_ONE:W_ONE + 64]

    fc1b = fp_[0:16, F_B + 0:F_B + 1]
    fc2b = fp_[0:2, F_B + 1:F_B + 2]
    fc3b = fp_[0:64, F_B + 2:F_B + 3]
    epsc128 = fp_[0:128, F_EPS:F_EPS + 1]
    xng_c = fp_[0:64, F_XG:F_XG + 1]
    xb3_c = fp_[0:64, F_XB3:F_XB3 + 1]

    # sigmoid-set table prefetch, ASAP on the scalar queue
    warm1 = small.tile([1, 1], FP, tag="warm1")
    nc.scalar.activation(warm1[:], fp_[0:1, F_EPS:F_EPS + 1], AF.Sigmoid)

    # ---------------- big persistent tiles ----------------
    xT = persist.tile([64, N], BF, tag="xT")
    lastT = persist.tile([64, N], BF, tag="lastT")
    c1 = persist.tile([128, N], BF, tag="c1")      # [x3 ; last]
    c2 = persist.tile([128, N], BF, tag="c2")      # [r*last ; x3]
    hgsq = persist.tile([128, N], BF, tag="hgsq")  # [Hg_raw ; Hg_raw^2]
    HgQ = persist.tile([66, N], BF, tag="HgQ")     # [Hg*a ; c ; 1]
    osq = persist.tile([128, HALF], BF, tag="osq")  # [origT ; origT^2]
    x2a = persist.tile([3, N], BF, tag="x2a")
    a_row = persist.tile([1, N], BF, tag="a_row")
    kT0 = persist.tile([128, N], BF, tag="kT0")
    kT1 = persist.tile([128, N], BF, tag="kT1")
    k2T0 = persist.tile([128, N], BF, tag="k2T0")
    k2T1 = persist.tile([128, N], BF, tag="k2T1")
    qT0 = persist.tile([128, HALF], BF, tag="qT0")
    qT1 = persist.tile([128, HALF], BF, tag="qT1")
    x3gs = persist.tile([67, N], BF, tag="x3gs")   # [x3*gs ; ga ; gs ; cb]
    x3rA = persist.tile([67, HALF], BF, tag="x3rA")
    ga_b = persist.tile([128, N], BF, tag="ga_b")
    gs_b = persist.tile([64, N], BF, tag="gs_b")
    ga_r = persist.tile([1, N], BF, tag="ga_r")
    gs_r = persist.tile([1, N], BF, tag="gs_r")
    gt_sb = persist.tile([128, 256], BF, tag="gt_sb")
    gb_sb = persist.tile([128, 256], BF, tag="gb_sb")
    gs_f = persist.tile([64, 64], BF, tag="gs_f")
    ks0 = persist.tile([128, 1], BF, tag="ks0")
    ks1 = persist.tile([128, 1], BF, tag="ks1")
    xsb = persist.tile([64, 1], BF, tag="xsb")
    rc32 = persist.tile([128, 4 * HCH], FP, tag="rc32")
    dg8 = persist.tile([128, HCH], FP, tag="dg8")
    e0sb = persist.tile([128, HALF], BF, tag="e0sb")
    e1sb = persist.tile([128, HALF], BF, tag="e1sb")
    essb = persist.tile([64, HALF], BF, tag="essb")
    ph1sb = persist.tile([64, HALF], BF, tag="ph1sb")
    finsq = persist.tile([128, HALF], BF, tag="finsq")
    cT_sb = persist.tile([128, NCH], FP, tag="cT_sb")
    lastR = persist.tile([128, HCH * 64], FP, tag="lastR")
    x1aug = persist.tile([65, HALF], BF, tag="x1aug")  # [xo^T + b3 ; 1]
    hca = persist.tile([65, HALF], BF, tag="hca")
    hcb = persist.tile([65, HALF], BF, tag="hcb")
    fin = persist.tile([128, HCH * 64], FP, tag="fin")

    # input loads
    nc.sync.dma_start(xT[:], io["xT"][:])
    nc.sync.dma_start(lastT[:], io["lastT"][:])
    nc.sync.dma_start(c1[64:128, :], io["lastT"][:])
    nc.gpsimd.dma_start(ga_r[:], io["corr4"][0:1, :])
    nc.gpsimd.dma_start(gs_r[:], io["corr4"][1:2, :])
    nc.gpsimd.dma_start(x3gs[64:67, :], io["corr4"][0:3, :])
    nc.scalar.dma_start(osq[0:64, :], io["origT"][:])
    # constant-ones rows
    nc.gpsimd.dma_start(HgQ[65:66, :], io["corr4"][3:4, :])
    nc.gpsimd.dma_start(x1aug[64:65, :], io["corr4"][3:4, 0:HALF])
    nc.gpsimd.dma_start(x2a[2:3, :], io["corr4"][3:4, :])
    nc.gpsimd.dma_start(hca[64:65, :], io["corr4"][3:4, 0:HALF])
    nc.gpsimd.dma_start(hcb[64:65, :], io["corr4"][3:4, 0:HALF])

    # LN parameter rows -> [128, 64] broadcast tiles via gpsimd (small)
    brows = {}
    for k, nm in enumerate(("bng", "bnb", "lng", "lnb")):
        t = persist.tile([128, 64], FP, tag=f"{nm}_b", name=f"{nm}_b")
        nc.gpsimd.partition_broadcast(
            t[:], fp_[0:1, F_BN + 64 * k:F_BN + 64 * (k + 1)])
        brows[nm] = t

    frontA = ExitStack()
    fps = frontA.enter_context(tc.tile_pool(name="fps", bufs=5, space="PSUM"))

    MBs = [slice(j * MB, (j + 1) * MB) for j in range(NMB)]
    HBs = [slice(j * MB, (j + 1) * MB) for j in range(2)]

    # ============ hyper fc stack (breadth-first stages) ============
    x1T = persist.tile([16, N], BF, tag="x1T")
    xacc = small.tile([64, NMB], FP, tag="xacc")
    p1 = [fps.tile([16, MB], FP, tag="fp", name=f"p1_{j}") for j in range(NMB)]
    for j in range(NMB):
        nc.tensor.matmul(p1[j][:], fc1s, xT[:, MBs[j]], start=True, stop=True)
    for j in range(NMB):
        nc.scalar.activation(x1T[:, MBs[j]], p1[j][:], AF.Sigmoid, bias=fc1b)
    p2 = [fps.tile([2, MB], FP, tag="fp", name=f"p2_{j}") for j in range(NMB)]
    for j in range(NMB):
        nc.tensor.matmul(p2[j][:], fc2s, x1T[:, MBs[j]], start=True, stop=True)
    for j in range(NMB):
        nc.scalar.activation(x2a[0:2, MBs[j]], p2[j][:], AF.Sigmoid, bias=fc2b)
    p3 = [fps.tile([64, MB], FP, tag="fp", name=f"p3_{j}") for j in range(NMB)]
    for j in range(NMB):
        nc.tensor.matmul(p3[j][:], fc3s, x2a[0:2, MBs[j]], start=True, stop=True)
    for j in range(NMB):
        nc.scalar.activation(c1[0:64, MBs[j]], p3[j][:], AF.Identity, bias=fc3b,
                             accum_out=xacc[:, j:j + 1])
    for j in range(NMB):
        nc.vector.tensor_copy(c2[64:128, MBs[j]], c1[0:64, MBs[j]])
    xs_f = small.tile([64, 1], FP, tag="xs_f")
    nc.vector.tensor_reduce(xs_f[:], xacc[:], AX.X, AL.add)
    nc.vector.tensor_copy(xsb[:], xs_f[:])

    # ================= GRU gates (breadth-first stages) =================
    gw = frontA.enter_context(tc.tile_pool(name="gw", bufs=4))
    zp = [fps.tile([64, MB], FP, tag="fp", name=f"zp_{j}") for j in range(NMB)]
    for j in range(NMB):
        nc.tensor.matmul(zp[j][:], wz, c1[:, MBs[j]], start=True, stop=True)
    zt = [gw.tile([64, MB], BF, tag="zt", name=f"zt_{j}") for j in range(NMB)]
    for j in range(NMB):
        nc.scalar.activation(zt[j][:], zp[j][:], AF.Sigmoid)
    rp = [fps.tile([64, MB], FP, tag="fp", name=f"rp_{j}") for j in range(NMB)]
    for j in range(NMB):
        nc.tensor.matmul(rp[j][:], wr, c1[:, MBs[j]], start=True, stop=True)
    rt = [gw.tile([64, MB], BF, tag="rt", name=f"rt_{j}") for j in range(NMB)]
    for j in range(NMB):
        nc.scalar.activation(rt[j][:], rp[j][:], AF.Sigmoid)
    for j in range(NMB):
        nc.vector.tensor_tensor(c2[0:64, MBs[j]], rt[j][:], lastT[:, MBs[j]], AL.mult)
    hp = [fps.tile([64, MB], FP, tag="fp", name=f"hp_{j}") for j in range(NMB)]
    for j in range(NMB):
        nc.tensor.matmul(hp[j][:], wh, c2[:, MBs[j]], start=True, stop=True)
    ht = [gw.tile([64, MB], BF, tag="ht", name=f"ht_{j}") for j in range(NMB)]
    for j in range(NMB):
        nc.scalar.activation(ht[j][:], hp[j][:], AF.Tanh)
    # switch the scalar act table to the sqrt set (the only switch)
    warm2 = small.tile([1, 1], FP, tag="warm2")
    nc.scalar.activation(warm2[:], fp_[0:1, F_EPS:F_EPS + 1], AF.Sqrt)
    dt_ = [gw.tile([64, MB], BF, tag="dt", name=f"dt_{j}") for j in range(NMB)]
    for j in range(NMB):
        nc.vector.tensor_tensor(dt_[j][:], ht[j][:], lastT[:, MBs[j]], AL.subtract)
    for j in range(NMB):
        nc.vector.tensor_tensor(dt_[j][:], dt_[j][:], zt[j][:], AL.mult)
    for j in range(NMB):
        nc.vector.tensor_tensor(hgsq[0:64, MBs[j]], dt_[j][:], lastT[:, MBs[j]], AL.add)
    for j in range(NMB):
        nc.vector.tensor_tensor(hgsq[64:128, MBs[j]], hgsq[0:64, MBs[j]],
                                hgsq[0:64, MBs[j]], AL.mult)

    # ---- xo squares (input-only dependent) ----
    for j in range(2):
        nc.vector.tensor_tensor(osq[64:128, HBs[j]], osq[0:64, HBs[j]],
                                osq[0:64, HBs[j]], AL.mult)

    # ============ xo per-chunk stats -> oa/oc rows ============
    oxst = fps.tile([128, 2 * HCH], FP, tag="fp", name="oxst",
                    padded_shape=[128, 512])
    for ci in range(HCH):
        csl = slice(ci * 128, (ci + 1) * 128)
        nc.tensor.matmul(oxst[:, 2 * ci:2 * ci + 2], osq[:, csl], sel2,
                         start=True, stop=True, skip_group_check=True)
    ox3 = oxst[:].rearrange("p (c s) -> p s c", s=2)
    omu = small.tile([128, HCH], FP, tag="omu")
    nc.vector.tensor_scalar(omu[:].unsqueeze(1), ox3[:, 0:1, :], 1.0 / 64,
                            None, AL.mult)
    om2 = small.tile([128, HCH], FP, tag="om2")
    nc.vector.tensor_tensor(om2[:], omu[:], omu[:], AL.mult)
    ovar = small.tile([128, HCH], FP, tag="ovar")
    nc.vector.scalar_tensor_tensor(ovar[:].unsqueeze(1), ox3[:, 1:2, :], 1.0 / 64,
                                   om2[:].unsqueeze(1), AL.mult, AL.subtract)
    osd = small.tile([128, HCH], FP, tag="osd")
    nc.scalar.activation(osd[:], ovar[:], AF.Sqrt, bias=epsc128)
    oa = small.tile([128, HCH], FP, tag="oa")
    nc.vector.reciprocal(oa[:], osd[:])
    opack = small.tile([128, 2 * HCH], BF, tag="opack")
    nc.vector.tensor_copy(opack[:, 0:HCH], oa[:])
    nc.vector.scalar_tensor_tensor(opack[:, HCH:2 * HCH], omu[:], -1.0, oa[:],
                                   AL.mult, AL.mult)
    otp = fps.tile([2 * HCH, 128], BF, tag="fp", name="otp",
                   padded_shape=[16, 1024])
    _tp(nc, otp[:], opack[:], identb)
    oT = small.tile([2 * HCH, 128], BF, tag="oT")
    nc.vector.tensor_copy(oT[:], otp[:])
    oar = small.tile([1, HALF], BF, tag="oar")
    nc.sync.dma_start(oar[:].rearrange("o (i p) -> o i p", p=128), oT[0:HCH, :])
    ocr = small.tile([1, HALF], BF, tag="ocr")
    nc.gpsimd.dma_start(ocr[:].rearrange("o (i p) -> o i p", p=128),
                        oT[HCH:2 * HCH, :])

    # ============ Hg per-chunk stats -> a_row / c row / cT_sb ============
    hst = fps.tile([128, 2 * NCH], FP, tag="fp", name="hst",
                   padded_shape=[128, 512])
    for ci in range(NCH):
        csl = slice(ci * 128, (ci + 1) * 128)
        nc.tensor.matmul(hst[:, 2 * ci:2 * ci + 2], hgsq[:, csl], sel2,
                         start=True, stop=True, skip_group_check=True)
    h3 = hst[:].rearrange("p (c s) -> p s c", s=2)
    hmu = small.tile([128, NCH], FP, tag="hmu")
    nc.vector.tensor_scalar(hmu[:].unsqueeze(1), h3[:, 0:1, :], 1.0 / 64,
                            None, AL.mult)
    hm2 = small.tile([128, NCH], FP, tag="hm2")
    nc.vector.tensor_tensor(hm2[:], hmu[:], hmu[:], AL.mult)
    hvar = small.tile([128, NCH], FP, tag="hvar")
    nc.vector.scalar_tensor_tensor(hvar[:].unsqueeze(1), h3[:, 1:2, :], 1.0 / 64,
                                   hm2[:].unsqueeze(1), AL.mult, AL.subtract)
    hsd = small.tile([128, NCH], FP, tag="hsd")
    nc.scalar.activation(hsd[:], hvar[:], AF.Sqrt, bias=epsc128)
    ha = small.tile([128, NCH], FP, tag="ha")
    nc.vector.reciprocal(ha[:], hsd[:])
    # cT_sb = -mu/sd in chunk layout (lastH bias), fp32
    nc.vector.scalar_tensor_tensor(cT_sb[:], hmu[:], -1.0, ha[:],
                                   AL.mult, AL.mult)
    hpack = small.tile([128, 2 * NCH], BF, tag="hpack")
    nc.vector.tensor_copy(hpack[:, 0:NCH], ha[:])
    nc.vector.tensor_copy(hpack[:, NCH:2 * NCH], cT_sb[:])
    htp = fps.tile([2 * NCH, 128], BF, tag="fp", name="htp",
                   padded_shape=[32, 1024])
    _tp(nc, htp[:], hpack[:], identb)
    haT = small.tile([2 * NCH, 128], BF, tag="haT")
    nc.vector.tensor_copy(haT[:], htp[:])
    nc.sync.dma_start(a_row[:].rearrange("o (i p) -> o i p", p=128), haT[0:NCH, :])
    nc.gpsimd.dma_start(HgQ[64:65, :].rearrange("o (i p) -> o i p", p=128),
                        haT[NCH:2 * NCH, :])

    # ---- xo affine into x1aug (oab/ocb broadcasts ready by now) ----
    oab = [fps.tile([64, MB], FP, tag="fp", name=f"oab_{j}") for j in range(2)]
    for j in range(2):
        nc.tensor.matmul(oab[j][:], onesr64, oar[:, HBs[j]], start=True, stop=True)
    ocb = [fps.tile([64, MB], FP, tag="fp", name=f"ocb_{j}") for j in range(2)]
    for j in range(2):
        nc.tensor.matmul(ocb[j][:], onesr64, ocr[:, HBs[j]], start=True, stop=True)
    for j in range(2):
        tb = small.tile([64, MB], BF, tag=f"oxt_{j}", name=f"oxt_{j}")
        nc.vector.tensor_tensor(tb[:], osq[0:64, HBs[j]], oab[j][:], AL.mult)
        nc.vector.tensor_tensor(tb[:], tb[:], ocb[j][:], AL.add)
        nc.scalar.activation(x1aug[0:64, HBs[j]], tb[:], AF.Identity,
                             scale=xng_c, bias=xb3_c)

    # ---- HgA = Hg_raw * a ----
    ab = [fps.tile([64, MB], FP, tag="fp", name=f"ab_{j}") for j in range(NMB)]
    for j in range(NMB):
        nc.tensor.matmul(ab[j][:], onesr64, a_row[:, MBs[j]], start=True, stop=True)
    for j in range(NMB):
        nc.vector.tensor_tensor(HgQ[0:64, MBs[j]], hgsq[0:64, MBs[j]], ab[j][:],
                                AL.mult)

    # ========== q/k projections + Gram matrices (interleaved on PE) ==========
    kacc = small.tile([128, 8], FP, tag="kacc")
    kjobs = []
    for half, dst in ((0, kT0), (1, kT1)):
        for j in range(NMB):
            kjobs.append((dst, slice(W_KA + 128 * half, W_KA + 128 * (half + 1)),
                          MBs[j], kacc[:, 4 * half + j:4 * half + j + 1]))
    qjobs = []
    for half, dst in ((0, qT0), (1, qT1)):
        for j in range(2):
            qjobs.append((dst, slice(W_QA + 128 * half, W_QA + 128 * (half + 1)),
                          HBs[j], None))
    jobs = kjobs + qjobs
    kq_ps = {}

    def leaky_evac(dst_ap, src_ap, acc, on_scalar):
        # PSUM has one DVE read port, so the vector path must evacuate
        # first and apply the leaky in place on SBUF.
        if on_scalar:
            if acc is not None:
                nc.scalar.activation(dst_ap, src_ap, AF.Prelu, alpha=0.01,
                                     accum_out=acc)
            else:
                nc.scalar.activation(dst_ap, src_ap, AF.Prelu, alpha=0.01)
        else:
            nc.vector.tensor_copy(dst_ap, src_ap)
            if acc is not None:
                nc.vector.scalar_tensor_tensor(dst_ap, dst_ap, 0.01, dst_ap,
                                               AL.mult, AL.max, accum_out=acc)
            else:
                nc.vector.scalar_tensor_tensor(dst_ap, dst_ap, 0.01, dst_ap,
                                               AL.mult, AL.max)

    gt_ps = fps.tile([128, 256], FP, tag="g", name="gt_ps", padded_shape=[128, 512], bufs=3)
    gb_ps = fps.tile([128, 256], FP, tag="g", name="gb_ps", padded_shape=[128, 512], bufs=3)
    gs_ps = fps.tile([64, 64], FP, tag="g", name="gs_ps", padded_shape=[64, 512], bufs=3)
    krp = frontA.enter_context(tc.tile_pool(name="krp", bufs=3))
    krs, xrs = {}, {}

    def gram_accum(g):
        st, sp = (g == 0), (g == NCH - 1)
        nc.tensor.matmul(gt_ps[:], krs[g][:, 0:128], krs[g][:], start=st, stop=sp)
        nc.tensor.matmul(gb_ps[:], krs[g][:, 128:256], krs[g][:], start=st, stop=sp)
        nc.tensor.matmul(gs_ps[:], xrs[g][:], xrs[g][:], start=st, stop=sp)

    for mi in range(NCH):
        msl = slice(mi * 128, (mi + 1) * 128)
        krq = fps.tile([128, 256], FP, tag="fp", name=f"krq_{mi}",
                       padded_shape=[128, 512])
        nc.tensor.matmul(krq[:], HgQ[:, msl], kA, start=True, stop=True)
        xrq = fps.tile([128, 64], FP, tag="fp", name=f"xrq_{mi}",
                       padded_shape=[128, 512])
        nc.tensor.matmul(xrq[:], x2a[:, msl], fc3a, start=True, stop=True)
        if mi < 12:
            dst, wsl, sl, acc = jobs[mi]
            kp = fps.tile([128, MB], FP, tag="fp", name=f"kqp_{mi}")
            nc.tensor.matmul(kp[:], wp[0:66, wsl], HgQ[:, sl], start=True, stop=True)
            kq_ps[mi] = kp
        kr = krp.tile([128, 256], BF, tag="kr", name=f"kr_{mi}")
        leaky_evac(kr[:], krq[:], None, mi % 2 == 0)
        xr = krp.tile([128, 64], BF, tag="xr", name=f"xr_{mi}")
        nc.vector.tensor_copy(xr[:], xrq[:])
        krs[mi], xrs[mi] = kr, xr
        if mi < 12:
            dst, wsl, sl, acc = jobs[mi]
            leaky_evac(dst[:, sl], kq_ps[mi][:], acc, mi % 2 == 1)
        if mi >= 2:
            gram_accum(mi - 2)
    gram_accum(NCH - 2)
    gram_accum(NCH - 1)
    nc.vector.tensor_copy(gt_sb[:], gt_ps[:])
    nc.vector.tensor_copy(gb_sb[:], gb_ps[:])
    nc.vector.tensor_copy(gs_f[:], gs_ps[:])
    ks_f = small.tile([128, 2], FP, tag="ks_f")
    nc.vector.tensor_reduce(ks_f[:], kacc[:].rearrange("p (h j) -> p h j", j=4),
                            AX.X, AL.add)
    nc.vector.tensor_copy(ks0[:], ks_f[:, 0:1])
    nc.vector.tensor_copy(ks1[:], ks_f[:, 1:2])

    # ga/gs broadcast tiles + k2 / x3gs
    gps_ = [fps.tile([128, MB], FP, tag="fp", name=f"gab_{j}") for j in range(NMB)]
    for j in range(NMB):
        nc.tensor.matmul(gps_[j][:], onesr128, ga_r[:, MBs[j]], start=True, stop=True)
    for j in range(NMB):
        nc.vector.tensor_copy(ga_b[:, MBs[j]], gps_[j][:])
    gss_ = [fps.tile([64, MB], FP, tag="fp", name=f"gsb_{j}") for j in range(NMB)]
    for j in range(NMB):
        nc.tensor.matmul(gss_[j][:], onesr64, gs_r[:, MBs[j]], start=True, stop=True)
    for j in range(NMB):
        nc.vector.tensor_copy(gs_b[:, MBs[j]], gss_[j][:])
    nc.vector.tensor_tensor(k2T0[:], kT0[:], ga_b[:], AL.mult)
    nc.vector.tensor_tensor(k2T1[:], kT1[:], ga_b[:], AL.mult)
    nc.vector.tensor_tensor(x3gs[0:64, :], c1[0:64, :], gs_b[:], AL.mult)

    # GCN layer-1 matmul (dl-independent)
    for jb in range(2):
        ph1 = fps.tile([64, MB], FP, tag="fp", name=f"ph1_{jb}")
        nc.tensor.matmul(ph1[:], w1a, x1aug[:, HBs[jb]], start=True, stop=True)
        nc.vector.tensor_copy(ph1sb[:, HBs[jb]], ph1[:])

    # lastH output (PE filler between kq and gram; needs HgA + cT_sb only)
    for i in range(HCH):
        pt = fps.tile([128, 64], BF, tag="fp", name=f"lpt_{i}",
                      padded_shape=[128, 1024])
        _tp(nc, pt[:], HgQ[0:64, i * 128:(i + 1) * 128], identb)
        nc.scalar.activation(lastR[:, i * 64:(i + 1) * 64], pt[:], AF.Identity,
                             bias=cT_sb[:, i:i + 1])
    l3 = lastR[:].rearrange("p (g e) -> p g e", e=64)
    lg3 = brows["bng"][:].unsqueeze(1).broadcast_to([128, HCH, 64])
    lb3 = brows["bnb"][:].unsqueeze(1).broadcast_to([128, HCH, 64])
    nc.vector.tensor_tensor(l3, l3, lg3, AL.mult)
    nc.vector.tensor_tensor(l3, l3, lb3, AL.add)

    # s1/t1 stat matmuls (only need q/c1/ks/xsb; overlap with row stats)
    sst1 = fps.tile([128, 16], FP, tag="fp", name="sst1", padded_shape=[128, 512])
    for ci in range(HCH):
        csl = slice(ci * 128, (ci + 1) * 128)
        nc.tensor.matmul(sst1[:, ci:ci + 1], qT0[:, csl], ks0[:],
                         start=True, stop=False, skip_group_check=True)
        nc.tensor.matmul(sst1[:, ci:ci + 1], qT1[:, csl], ks1[:],
                         start=False, stop=True, skip_group_check=True)
        nc.tensor.matmul(sst1[:, 8 + ci:9 + ci], c1[0:64, csl], xsb[:],
                         start=True, stop=True, skip_group_check=True)
    sstc1 = small.tile([128, 16], FP, tag="sstc1")
    nc.vector.tensor_copy(sstc1[:], sst1[:])

    frontA.close()

    # ========== own-row stats (transposed landing, no DMA scatter) ==========
    statq = ExitStack()
    ups = statq.enter_context(tc.tile_pool(name="ups", bufs=2, space="PSUM"))
    sps = statq.enter_context(tc.tile_pool(name="sps", bufs=1, space="PSUM"))
    for jb in range(2):
        sl = HBs[jb]
        ut0 = ups.tile([128, MB], FP, tag="ut", name=f"ut0_{jb}")
        nc.tensor.matmul(ut0[:], gt_sb[:, 0:128], qT0[:, sl], start=True, stop=False)
        nc.tensor.matmul(ut0[:], gb_sb[:, 0:128], qT1[:, sl], start=False, stop=True)
        ut1 = ups.tile([128, MB], FP, tag="ut", name=f"ut1_{jb}")
        nc.tensor.matmul(ut1[:], gt_sb[:, 128:256], qT0[:, sl], start=True, stop=False)
        nc.tensor.matmul(ut1[:], gb_sb[:, 128:256], qT1[:, sl], start=False, stop=True)
        nc.vector.tensor_tensor(e0sb[:, sl], ut0[:], qT0[:, sl], AL.mult)
        nc.vector.tensor_tensor(e1sb[:, sl], ut1[:], qT1[:, sl], AL.mult)
    for jb in range(2):
        sl = HBs[jb]
        us = ups.tile([64, MB], FP, tag="ut", name=f"us_{jb}")
        nc.tensor.matmul(us[:], gs_f[:], c1[0:64, sl], start=True, stop=True)
        nc.vector.tensor_tensor(essb[:, sl], us[:], c1[0:64, sl], AL.mult)

    # s2/t2 in one packed PSUM bank
    sst2 = sps.tile([128, 16], FP, tag="sst", padded_shape=[128, 512])
    for ci in range(HCH):
        csl = slice(ci * 128, (ci + 1) * 128)
        nc.tensor.matmul(sst2[:, ci:ci + 1], e0sb[:, csl], ones128c,
                         start=True, stop=False, skip_group_check=True)
        nc.tensor.matmul(sst2[:, ci:ci + 1], e1sb[:, csl], ones128c,
                         start=False, stop=True, skip_group_check=True)
        nc.tensor.matmul(sst2[:, 8 + ci:9 + ci], essb[:, csl], ones64c,
                         start=True, stop=True, skip_group_check=True)
    sstc2 = small.tile([128, 16], FP, tag="sstc2")
    nc.vector.tensor_copy(sstc2[:], sst2[:])
    statq.close()

    # ===== phase I pools (all 8 banks) =====
    zstack = ExitStack()
    zps = zstack.enter_context(tc.tile_pool(name="zps", bufs=8, space="PSUM"))
    scrv = zstack.enter_context(tc.tile_pool(name="scrv", bufs=2))
    scra = zstack.enter_context(tc.tile_pool(name="scra", bufs=2))
    ztiles = {}

    def passes12(i):
        csl = slice(i * 128, (i + 1) * 128)
        zpt = [zps.tile([128, MB], FP, tag="zpt", name=f"zp_{i}_{m}")
               for m in range(NMB)]
        ztiles[i] = zpt
        for mb in range(NMB):
            nc.tensor.matmul(zpt[mb][:], qT0[:, csl],
                             k2T0[:, mb * MB:(mb + 1) * MB],
                             start=True, stop=False)
        for mb in range(NMB):
            nc.tensor.matmul(zpt[mb][:], qT1[:, csl],
                             k2T1[:, mb * MB:(mb + 1) * MB],
                             start=False, stop=False)

    # fill the PE while the own-row stats land
    passes12(0)
    passes12(1)

    # padded sources for XBAR DMA-transposes (free dim must be 128)
    rpk = small.tile([128, 128], BF, tag="rpk")
    nc.vector.memset(rpk[:, 32:128], 0.0)
    dlpad = small.tile([128, 128], BF, tag="dlpad")
    nc.vector.memset(dlpad[:], 0.0)
    dlT = small.tile([128, 128], BF, tag="dlT")

    # ---- stat landing math (vector/scalar on [128,8] groups) ----
    smu = small.tile([128, 16], FP, tag="smu")
    nc.vector.tensor_scalar(smu[:], sstc1[:], 1.0 / N, None, AL.mult)
    sm2 = small.tile([128, 16], FP, tag="sm2")
    nc.vector.tensor_tensor(sm2[:], smu[:], smu[:], AL.mult)
    svar = small.tile([128, 16], FP, tag="svar")
    nc.vector.scalar_tensor_tensor(svar[:], sstc2[:], 1.0 / N, sm2[:],
                                   AL.mult, AL.subtract)
    ssd = small.tile([128, 16], FP, tag="ssd")
    nc.scalar.activation(ssd[:], svar[:], AF.Sqrt, bias=epsc128)
    rsS = small.tile([128, 8], FP, tag="rsS")
    nc.vector.reciprocal(rsS[:], ssd[:, 8:16])
    rho = small.tile([128, 8], FP, tag="rho")
    nc.vector.tensor_tensor(rho[:], ssd[:, 0:8], rsS[:], AL.mult)
    # rho lands first: its broadcast chain is the longest pole to pass3
    nc.vector.tensor_copy(rpk[:, 24:32], rho[:])
    nc.vector.tensor_scalar(rpk[:, 0:8], smu[:, 0:8], -1.0, None, AL.mult)
    nc.vector.scalar_tensor_tensor(rpk[:, 8:16], smu[:, 8:16], -1.0, rho[:],
                                   AL.mult, AL.mult)
    nc.vector.tensor_copy(rpk[:, 16:24], ssd[:, 0:8])
    rT = small.tile([128, 128], BF, tag="rT")
    nc.sync.dma_start(rT[:], rpk[:], transpose=True)
    rho_row = small.tile([1, HALF], BF, tag="rho_row")
    nc.gpsimd.dma_start(rho_row[:].rearrange("o (i p) -> o i p", p=128),
                        rT[24:32, :])
    x3rh = small.tile([64, HALF], BF, tag="x3rh")
    nc.gpsimd.partition_broadcast(x3rh[:], rho_row[:])
    nc.vector.tensor_tensor(x3rA[0:64, :], c1[0:64, 0:HALF], x3rh[:], AL.mult)
    for r, eng in ((0, nc.sync), (1, nc.scalar), (2, nc.sync)):
        eng.dma_start(
            x3rA[64 + r:65 + r, :].rearrange("o (i p) -> o i p", p=128),
            rT[8 * r:8 * r + 8, :])
    # lastH store, emitted late so it cannot head-of-line block the land
    nc.sync.dma_start(io["lastH"].rearrange("(i p) e -> p i e", p=128),
                      lastR[:].rearrange("p (i e) -> p i e", e=64))

    def pass3(i):
        csl = slice(i * 128, (i + 1) * 128)
        zpt = ztiles[i]
        for mb in range(NMB):
            nc.tensor.matmul(zpt[mb][:], x3rA[:, csl],
                             x3gs[:, mb * MB:(mb + 1) * MB],
                             start=False, stop=True)
        for mb in range(NMB):
            acc = rc32[:, 4 * i + mb:4 * i + mb + 1]
            if mb % 2 == 0:
                scr = scrv.tile([128, MB], FP, tag="scr", name=f"scr_{i}_{mb}")
                nc.vector.tensor_scalar(scr[:], zpt[mb][:], 0.0, None, AL.max,
                                        AL.add, accum_out=acc)
            else:
                scr = scra.tile([128, MB], FP, tag="scr2", name=f"scr2_{i}_{mb}")
                nc.scalar.activation(scr[:], zpt[mb][:], AF.Relu, accum_out=acc)
            if mb == i // 4:
                off = (i * 128) % MB
                dsel = scrv.tile([128, 128], FP, tag="dsel", name=f"dsel_{i}")
                nc.gpsimd.affine_select(
                    out=dsel[:], in_=scr[:, off:off + 128],
                    compare_op=AL.is_equal, fill=0.0, base=0,
                    pattern=[[-1, 128]], channel_multiplier=1)
                nc.vector.tensor_reduce(dg8[:, i:i + 1], dsel[:], AX.X, AL.add)

    # -------- per-pair GCN/diag/output tail, split into 3 stages --------
    # T1(p): diag/rowsum -> dls broadcast + GCN-2 input (no PE, no PSUM)
    # T2(p): GCN layers 2/3 + square (2 PSUM tiles, emitted one chunk later)
    # T3(p): final LN stats + transpose + output DMA (3 PSUM tiles)
    def tail1(p):
        c0 = 2 * p
        psl = slice(256 * p, 256 * (p + 1))
        rs2 = small.tile([128, 2], FP, tag=f"rs2_{p}", name=f"rs2_{p}")
        nc.vector.tensor_reduce(
            rs2[:], rc32[:, 8 * p:8 * p + 8].rearrange("p (i m) -> p i m", m=4),
            AX.X, AL.add)
        nc.vector.reciprocal(rs2[:], rs2[:])
        if p < 3:
            nc.vector.tensor_tensor(dlpad[:, c0:c0 + 2], dg8[:, c0:c0 + 2],
                                    rs2[:], AL.mult)
            nc.scalar.dma_start(dlT[:], dlpad[:], transpose=True)
            dl_src = dlT[c0:c0 + 2, :]
        else:
            # last pair: PSUM is free, use the short PE-transpose path
            dl2 = small.tile([128, 2], BF, tag="dl2_3", name="dl2_3")
            nc.vector.tensor_tensor(dl2[:], dg8[:, c0:c0 + 2], rs2[:], AL.mult)
            dltp = zps.tile([2, 128], BF, tag="zpt", name="dltp_3",
                            padded_shape=[2, 1024])
            _tp(nc, dltp[:], dl2[:], identb)
            dlT2 = small.tile([2, 128], BF, tag="dlT2_3", name="dlT2_3")
            nc.scalar.copy(dlT2[:], dltp[:])
            dl_src = dlT2[:]
        dlr = small.tile([1, 256], BF, tag=f"dlr_{p}", name=f"dlr_{p}")
        nc.sync.dma_start(dlr[:].rearrange("o (i p) -> o i p", p=128), dl_src)
        dls = small.tile([64, 256], BF, tag=f"dls_{p}", name=f"dls_{p}")
        nc.gpsimd.partition_broadcast(dls[:], dlr[:])
        nc.vector.tensor_tensor(hca[0:64, psl], ph1sb[:, psl], dls[:], AL.mult)
        return dls

    def tail2(p, dls):
        psl = slice(256 * p, 256 * (p + 1))
        ph2 = zps.tile([64, 256], FP, tag="zpt", name=f"ph2_{p}",
                       padded_shape=[64, 512])
        nc.tensor.matmul(ph2[:], w2a, hca[:, psl], start=True, stop=True)
        nc.vector.tensor_tensor(hcb[0:64, psl], ph2[:], dls[:], AL.mult)
        ph3 = zps.tile([64, 256], FP, tag="zpt", name=f"ph3_{p}",
                       padded_shape=[64, 512])
        nc.tensor.matmul(ph3[:], w3a, hcb[:, psl], start=True, stop=True)
        nc.vector.tensor_tensor(finsq[0:64, psl], ph3[:], dls[:], AL.mult)
        nc.vector.tensor_tensor(finsq[0:64, psl], finsq[0:64, psl],
                                x1aug[0:64, psl], AL.add)
        nc.scalar.square(finsq[64:128, psl], finsq[0:64, psl])

    def tail3(p):
        c0 = 2 * p
        fst = zps.tile([128, 4], FP, tag="zpt", name=f"fst_{p}",
                       padded_shape=[128, 512])
        for k in range(2):
            csl = slice((c0 + k) * 128, (c0 + k + 1) * 128)
            nc.tensor.matmul(fst[:, 2 * k:2 * k + 2], finsq[:, csl], sel2,
                             start=True, stop=True, skip_group_check=True)
        f3 = fst[:].rearrange("p (c s) -> p s c", s=2)
        fmu = small.tile([128, 2], FP, tag=f"fmu_{p}", name=f"fmu_{p}")
        nc.vector.tensor_scalar(fmu[:].unsqueeze(1), f3[:, 0:1, :], 1.0 / 64,
                                None, AL.mult)
        fm2 = small.tile([128, 2], FP, tag=f"fm2_{p}", name=f"fm2_{p}")
        nc.vector.tensor_tensor(fm2[:], fmu[:], fmu[:], AL.mult)
        fvar = small.tile([128, 2], FP, tag=f"fvar_{p}", name=f"fvar_{p}")
        nc.vector.scalar_tensor_tensor(fvar[:].unsqueeze(1), f3[:, 1:2, :],
                                       1.0 / 64, fm2[:].unsqueeze(1),
                                       AL.mult, AL.subtract)
        fsd = small.tile([128, 2], FP, tag=f"fsd_{p}", name=f"fsd_{p}")
        nc.scalar.activation(fsd[:], fvar[:], AF.Sqrt, bias=epsc128)
        fa = small.tile([128, 2], FP, tag=f"fa_{p}", name=f"fa_{p}")
        nc.vector.reciprocal(fa[:], fsd[:])
        fc = small.tile([128, 2], FP, tag=f"fc_{p}", name=f"fc_{p}")
        nc.vector.scalar_tensor_tensor(fc[:], fmu[:], -1.0, fa[:],
                                       AL.mult, AL.mult)
        for k in range(2):
            i = c0 + k
            ftp = zps.tile([128, 64], BF, tag="zpt", name=f"ftp_{i}",
                           padded_shape=[128, 1024])
            _tp(nc, ftp[:], finsq[0:64, i * 128:(i + 1) * 128], identb)
            nc.scalar.activation(fin[:, i * 64:(i + 1) * 64], ftp[:],
                                 AF.Identity, scale=fa[:, k:k + 1],
                                 bias=fc[:, k:k + 1])
        fpair = fin[:, 128 * p:128 * (p + 1)].rearrange("p (g e) -> p g e", e=64)
        fg3 = brows["lng"][:].unsqueeze(1).broadcast_to([128, 2, 64])
        fb3 = brows["lnb"][:].unsqueeze(1).broadcast_to([128, 2, 64])
        nc.vector.tensor_tensor(fpair, fpair, fg3, AL.mult)
        nc.vector.tensor_tensor(fpair, fpair, fb3, AL.add)
        nc.sync.dma_start(
            io["outH"][256 * p:256 * (p + 1), :].rearrange(
                "(i p) e -> p i e", p=128),
            fin[:, 128 * p:128 * (p + 1)].rearrange("p (i e) -> p i e", e=64))

    dls_of = {}
    for i in range(HCH):
        pass3(i)
        if i >= 2 and i % 2 == 0:
            tail2((i - 2) // 2, dls_of[(i - 2) // 2])
        if i >= 3 and i % 2 == 1:
            tail3((i - 3) // 2)
        if i + 2 < HCH:
            passes12(i + 2)
        if i % 2 == 1:
            dls_of[i // 2] = tail1(i // 2)
    tail2(3, dls_of[3])
    tail3(3)
    zstack.close()


def _build():
    if "nc" in _CACHE:
        return _CACHE["nc"]
    nc = bacc.Bacc("TRN2", target_bir_lowering=False, debug=False,
                   enable_asserts=True, num_devices=8)
    io = {}
    io["xT"] = nc.dram_tensor("xT", [G, N], BF, kind="ExternalInput").ap()
    io["lastT"] = nc.dram_tensor("lastT", [G, N], BF, kind="ExternalInput").ap()
    io["origT"] = nc.dram_tensor("origT", [E, HALF], BF, kind="ExternalInput").ap()
    io["corr4"] = nc.dram_tensor("corr4", [4, N], BF, kind="ExternalInput").ap()
    io["wpack"] = nc.dram_tensor("wpack", [128, WPACK_W], BF, kind="ExternalInput").ap()
    io["fpack"] = nc.dram_tensor("fpack", [128, FPACK_W], FP, kind="ExternalInput").ap()
    io["outH"] = nc.dram_tensor("outH", [HALF, E], FP, kind="ExternalOutput").ap()
    io["lastH"] = nc.dram_tensor("lastH", [HALF, G], FP, kind="ExternalOutput").ap()

    with tile.TileContext(nc) as tc:
        with ExitStack() as ctx:
            _emit(ctx, tc, io)
    nc.compile()
    nc.m = get_hw_module(nc.m)
    _CACHE["nc"] = nc
    return nc


def _host_prep(inputs):
    f32 = np.float32
    bf = ml_dtypes.bfloat16
    inp = {k: np.asarray(v, f32) for k, v in inputs.items()}
    ch = 1.0 + inp["mlp_w"].sum(axis=0)
    assert (ch > 0).all(), "head-mixing scale fold requires positive c_h"
    g, b = inp["bn_g"], inp["bn_b"]
    qw_c = inp["q_w"] * np.repeat(ch / np.sqrt(G), G)[None, :]
    Wq = g[:, None] * qw_c
    qA = np.concatenate([Wq, Wq.sum(axis=0)[None], (b @ qw_c)[None]], axis=0)
    Wk = g[:, None] * inp["k_w"]
    kA = np.concatenate([Wk, Wk.sum(axis=0)[None], (b @ inp["k_w"])[None]], axis=0)
    w1 = inp["gcn_w1"]
    w1a = np.concatenate([w1, -(inp["gcn_b3"] @ w1)[None]], axis=0)
    w2a = np.concatenate([inp["gcn_w2"], (inp["gcn_b1"] @ inp["gcn_w2"])[None]], axis=0)
    w3a = np.concatenate([inp["gcn_w3"], (inp["gcn_b2"] @ inp["gcn_w3"])[None]], axis=0)

    wpack = np.zeros((128, WPACK_W), f32)
    wpack[0:128, W_IDB:W_IDB + 128] = np.eye(128)
    wpack[0:128, W_WZ:W_WZ + 64] = inp["w_z"]
    wpack[0:128, W_WR:W_WR + 64] = inp["w_r"]
    wpack[0:128, W_WH:W_WH + 64] = inp["w_h"]
    wpack[0:66, W_QA:W_QA + 256] = qA
    wpack[0:66, W_KA:W_KA + 256] = kA
    wpack[0:64, W_FC1:W_FC1 + 16] = inp["fc1_w"]
    wpack[0:16, W_FC2:W_FC2 + 2] = inp["fc2_w"]
    wpack[0:2, W_FC3A:W_FC3A + 64] = inp["fc3_w"]
    wpack[2, W_FC3A:W_FC3A + 64] = inp["fc3_b"]
    wpack[0:65, W_W1A:W_W1A + 64] = w1a
    wpack[0:65, W_W2A:W_W2A + 64] = w2a
    wpack[0:65, W_W3A:W_W3A + 64] = w3a
    wpack[0:64, W_SEL:W_SEL + 1] = 1.0
    wpack[64:128, W_SEL + 1:W_SEL + 2] = 1.0
    wpack[:, W_ONE:W_ONE + 128] = 1.0

    fpack = np.zeros((128, FPACK_W), f32)
    fpack[0:16, F_B + 0] = inp["fc1_b"]
    fpack[0:2, F_B + 1] = inp["fc2_b"]
    fpack[0:64, F_B + 2] = inp["fc3_b"]
    fpack[0:128, F_EPS] = EPS
    fpack[0:64, F_XG] = inp["x_nom_g"]
    fpack[0:64, F_XB3] = inp["x_nom_b"] + inp["gcn_b3"]
    for k, nm in enumerate(("bn_g", "bn_b", "last_nom_g", "last_nom_b")):
        fpack[0, F_BN + 64 * k:F_BN + 64 * (k + 1)] = inp[nm]

    def c(a, dt=bf):
        return np.ascontiguousarray(np.asarray(a, dt))

    shared = {"wpack": c(wpack), "fpack": c(fpack, f32)}
    in_maps = []
    for core in range(8):
        bi, h = core // 2, core % 2
        off = h * HALF
        corr4 = np.stack([
            np.roll(inp["attn_norm_g"], -off),
            np.roll(inp["skip_norm_g"], -off),
            np.roll(inp["attn_norm_b"] + inp["skip_norm_b"], -off),
            np.ones(N, f32),
        ])
        m = dict(shared)
        m["xT"] = c(np.roll(inp["x"][bi], -off, axis=0).T)
        m["lastT"] = c(np.roll(inp["last_G_emb"][bi], -off, axis=0).T)
        m["origT"] = c(inp["orig_x"][bi, off:off + HALF].T)
        m["corr4"] = c(corr4)
        in_maps.append(m)
    return in_maps


def run(inputs, trace=False):
    nc = _build()
    in_maps = _host_prep(inputs)
    res = run_bass_kernel_spmd(nc, in_maps, core_ids=list(range(8)), trace=trace)
    out = np.zeros((B, N, E), np.float32)
    last = np.zeros((B, N, G), np.float32)
    for core in range(8):
        bi, h = core // 2, core % 2
        off = h * HALF
        out[bi, off:off + HALF] = res.results[core]["outH"]
        last[bi, off:off + HALF] = res.results[core]["lastH"]
    return (out, last), res


def kernel(**inputs):
    return run(inputs)[0]


# revision 27
# speedup vs baseline: 1.1970x; 1.1970x over previous
"""Trainium2 Bass kernel for nn_DGCN (gnn_message_passing).

Sharding: 8 shards = (batch b in 0..3, row-half h in 0..1). Each core gets
the full 2048-node K-side tensors of its batch with the node axis ROTATED
by h*1024 so the adjacency diagonal lands at the same tile position on
every core (uniform SPMD program); the core computes rows 0..1023 of the
rotated order, which are rows [h*1024, (h+1)*1024) of the original order.

v6 — latency-first restructure of v4:
 - All per-node LN statistics land directly in chunk-column layout via
   per-chunk transposed stat matmuls (lhsT = data chunk, rhs = selector),
   eliminating every SBUF->SBUF scatter DMA round trip; only single-row
   gathers remain (split across the sync/scalar/gpsimd queues), fed by an
   XBAR DMA-transpose of the packed [128,32] stat tile (no PSUM use, so
   the land cannot deadlock against the fully-committed phase-I banks).
 - leaky_relu fuses into the PSUM->SBUF evacuation via the scalar
   engine's Prelu activation (alpha=0.01; Prelu, unlike Lrelu, lives in
   21 act-table sets so it never forces a table switch). Scalar table
   switches are monotone: sigmoid-set ops (fc, GRU) precede the single
   switch to the sqrt set, each prefetched by a dummy op.
 - The q/k projections, the K-side/x3 Gram chunk projections (krq/xrq
   off HgQ/x2a — independent of the kq evacuations), and the Gram
   accumulations are interleaved on the PE so the whole mid-section is
   tensor-throughput-bound instead of chained; evacuations alternate
   scalar Prelu / vector copy+leaky. s1/t1 row-stat matmuls issue as
   soon as ks lands; s2/t2 follow the Gram. PSUM: 5 rotating front banks
   + 3 pinned Gram accumulator banks.
 - The GCN/diag tail is pipelined per 2-chunk pair inside phase I in 3
   stages (diag/rowsum+broadcast -> GCN-2/3 -> final LN + store), each
   emitted one chunk apart so tail PSUM allocs never overlap the 8
   committed zpt banks; after the final N^2 chunk only ~one pair's tail
   remains. The last pair uses a PE transpose (PSUM is free by then).
 - Phase I (the single fused N^2 pass: q.k2 + x3-Gram bracket, relu,
   rowsum via accum_out, diag via affine_select) is unchanged
   mathematically, software-pipelined two chunks deep across all 8 PSUM
   banks.
 - Known non-fix: the PE duty-cycles between 1.2 and 2.4 GHz run-to-run
   (device power state); wall time is bimodal ~142us / ~17xus across
   runs with identical instruction streams.
"""

import sys

if '/opt/trn_rl_repo' not in sys.path:
    sys.path.insert(0, '/opt/trn_rl_repo')

from contextlib import ExitStack

import numpy as np
import ml_dtypes

import concourse.bass as bass
import concourse.tile as tile
from concourse import bacc, mybir
from concourse.bass_interp import get_hw_module
from concourse.bass_utils import run_bass_kernel_spmd

FP = mybir.dt.float32
BF = mybir.dt.bfloat16
AL = mybir.AluOpType
AF = mybir.ActivationFunctionType
AX = mybir.AxisListType

B, N, E, G, H = 4, 2048, 64, 64, 4
D = H * G          # 256
HALF = N // 2      # own rows per core
NCH = N // 128     # 16 chunks over all nodes
HCH = HALF // 128  # 8 own chunks
MB = 512
NMB = N // MB      # 4
EPS = 1e-5

# wpack (bf16 [128, WPACK_W]) column layout
W_IDB, W_WZ, W_WR, W_WH = 0, 128, 192, 256
W_QA, W_KA = 320, 576
W_FC1, W_FC2, W_FC3A = 832, 848, 850
W_W1A, W_W2A, W_W3A = 914, 978, 1042
W_SEL, W_ONE = 1106, 1108
WPACK_W = 1280
# fpack (fp32 [128, FPACK_W]) column layout
F_B, F_EPS, F_XG, F_XB3, F_BN = 0, 3, 4, 5, 8
FPACK_W = 264

_CACHE = {}


def _tp(nc, out_ap, in_ap, ident):
    k = in_ap.partition_size()
    nc.tensor.transpose(out_ap, in_ap, ident[0:k, 0:k])


def _emit(ctx: ExitStack, tc: tile.TileContext, io: dict):
    nc = tc.nc

    persist = ctx.enter_context(tc.tile_pool(name="persist", bufs=1))
    small = ctx.enter_context(tc.tile_pool(name="small", bufs=1))

    # ---------------- packed params (2 DMAs) ----------------
    wp = persist.tile([128, WPACK_W], BF, tag="wp")
    nc.sync.dma_start(wp[:], io["wpack"][:])
    fp_ = persist.tile([128, FPACK_W], FP, tag="fp_")
    nc.scalar.dma_start(fp_[:], io["fpack"][:])

    identb = wp[:, W_IDB:W_IDB + 128]
    wz = wp[:, W_WZ:W_WZ + 64]
    wr = wp[:, W_WR:W_WR + 64]
    wh = wp[:, W_WH:W_WH + 64]
    fc1s = wp[0:64, W_FC1:W_FC1 + 16]
    fc2s = wp[0:16, W_FC2:W_FC2 + 2]
    fc3s = wp[0:2, W_FC3A:W_FC3A + 64]
    fc3a = wp[0:3, W_FC3A:W_FC3A + 64]
    kA = wp[0:66, W_KA:W_KA + 256]
    w1a = wp[0:65, W_W1A:W_W1A + 64]
    w2a = wp[0:65, W_W2A:W_W2A + 64]
    w3a = wp[0:65, W_W3A:W_W3A + 64]
    sel2 = wp[:, W_SEL:W_SEL + 2]
    ones128c = wp[:, W_ONE:W_ONE + 1]
    ones64c = wp[0:64, W_ONE:W_ONE + 1]
    onesr128 = wp[0:1, W_ONE:W_ONE + 128]
    onesr64 = wp[0:1, W_ONE:W_ONE + 64]

    fc1b = fp_[0:16, F_B + 0:F_B + 1]
    fc2b = fp_[0:2, F_B + 1:F_B + 2]
    fc3b = fp_[0:64, F_B + 2:F_B + 3]
    epsc128 = fp_[0:128, F_EPS:F_EPS + 1]
    xng_c = fp_[0:64, F_XG:F_XG + 1]
    xb3_c = fp_[0:64, F_XB3:F_XB3 + 1]

    # sigmoid-set table prefetch, ASAP on the scalar queue
    warm1 = small.tile([1, 1], FP, tag="warm1")
    nc.scalar.activation(warm1[:], fp_[0:1, F_EPS:F_EPS + 1], AF.Sigmoid)

    # ---------------- big persistent tiles ----------------
    xT = persist.tile([64, N], BF, tag="xT")
    lastT = persist.tile([64, N], BF, tag="lastT")
    c1 = persist.tile([128, N], BF, tag="c1")      # [x3 ; last]
    c2 = persist.tile([128, N], BF, tag="c2")      # [r*last ; x3]
    hgsq = persist.tile([128, N], BF, tag="hgsq")  # [Hg_raw ; Hg_raw^2]
    HgQ = persist.tile([66, N], BF, tag="HgQ")     # [Hg*a ; c ; 1]
    osq = persist.tile([128, HALF], BF, tag="osq")  # [origT ; origT^2]
    x2a = persist.tile([3, N], BF, tag="x2a")
    a_row = persist.tile([1, N], BF, tag="a_row")
    kT0 = persist.tile([128, N], BF, tag="kT0")
    kT1 = persist.tile([128, N], BF, tag="kT1")
    k2T0 = persist.tile([128, N], BF, tag="k2T0")
    k2T1 = persist.tile([128, N], BF, tag="k2T1")
    qT0 = persist.tile([128, HALF], BF, tag="qT0")
    qT1 = persist.tile([128, HALF], BF, tag="qT1")
    x3gs = persist.tile([67, N], BF, tag="x3gs")   # [x3*gs ; ga ; gs ; cb]
    x3rA = persist.tile([67, HALF], BF, tag="x3rA")
    ga_b = persist.tile([128, N], BF, tag="ga_b")
    gs_b = persist.tile([64, N], BF, tag="gs_b")
    ga_r = persist.tile([1, N], BF, tag="ga_r")
    gs_r = persist.tile([1, N], BF, tag="gs_r")
    gt_sb = persist.tile([128, 256], BF, tag="gt_sb")
    gb_sb = persist.tile([128, 256], BF, tag="gb_sb")
    gs_f = persist.tile([64, 64], BF, tag="gs_f")
    ks0 = persist.tile([128, 1], BF, tag="ks0")
    ks1 = persist.tile([128, 1], BF, tag="ks1")
    xsb = persist.tile([64, 1], BF, tag="xsb")
    rc32 = persist.tile([128, 4 * HCH], FP, tag="rc32")
    dg8 = persist.tile([128, HCH], FP, tag="dg8")
    e0sb = persist.tile([128, HALF], BF, tag="e0sb")
    e1sb = persist.tile([128, HALF], BF, tag="e1sb")
    essb = persist.tile([64, HALF], BF, tag="essb")
    ph1sb = persist.tile([64, HALF], BF, tag="ph1sb")
    finsq = persist.tile([128, HALF], BF, tag="finsq")
    cT_sb = persist.tile([128, NCH], FP, tag="cT_sb")
    lastR = persist.tile([128, HCH * 64], FP, tag="lastR")
    x1aug = persist.tile([65, HALF], BF, tag="x1aug")  # [xo^T + b3 ; 1]
    hca = persist.tile([65, HALF], BF, tag="hca")
    hcb = persist.tile([65, HALF], BF, tag="hcb")
    fin = persist.tile([128, HCH * 64], FP, tag="fin")

    # input loads
    nc.sync.dma_start(xT[:], io["xT"][:])
    nc.sync.dma_start(lastT[:], io["lastT"][:])
    nc.sync.dma_start(c1[64:128, :], io["lastT"][:])
    nc.gpsimd.dma_start(ga_r[:], io["corr4"][0:1, :])
    nc.gpsimd.dma_start(gs_r[:], io["corr4"][1:2, :])
    nc.gpsimd.dma_start(x3gs[64:67, :], io["corr4"][0:3, :])
    nc.scalar.dma_start(osq[0:64, :], io["origT"][:])
    # constant-ones rows
    nc.gpsimd.dma_start(HgQ[65:66, :], io["corr4"][3:4, :])
    nc.gpsimd.dma_start(x1aug[64:65, :], io["corr4"][3:4, 0:HALF])
    nc.gpsimd.dma_start(x2a[2:3, :], io["corr4"][3:4, :])
    nc.gpsimd.dma_start(hca[64:65, :], io["corr4"][3:4, 0:HALF])
    nc.gpsimd.dma_start(hcb[64:65, :], io["corr4"][3:4, 0:HALF])

    # LN parameter rows -> [128, 64] broadcast tiles via gpsimd (small)
    brows = {}
    for k, nm in enumerate(("bng", "bnb", "lng", "lnb")):
        t = persist.tile([128, 64], FP, tag=f"{nm}_b", name=f"{nm}_b")
        nc.gpsimd.partition_broadcast(
            t[:], fp_[0:1, F_BN + 64 * k:F_BN + 64 * (k + 1)])
        brows[nm] = t

    frontA = ExitStack()
    fps = frontA.enter_context(tc.tile_pool(name="fps", bufs=5, space="PSUM"))

    MBs = [slice(j * MB, (j + 1) * MB) for j in range(NMB)]
    HBs = [slice(j * MB, (j + 1) * MB) for j in range(2)]

    # ============ hyper fc stack (breadth-first stages) ============
    x1T = persist.tile([16, N], BF, tag="x1T")
    xacc = small.tile([64, NMB], FP, tag="xacc")
    p1 = [fps.tile([16, MB], FP, tag="fp", name=f"p1_{j}") for j in range(NMB)]
    for j in range(NMB):
        nc.tensor.matmul(p1[j][:], fc1s, xT[:, MBs[j]], start=True, stop=True)
    for j in range(NMB):
        nc.scalar.activation(x1T[:, MBs[j]], p1[j][:], AF.Sigmoid, bias=fc1b)
    p2 = [fps.tile([2, MB], FP, tag="fp", name=f"p2_{j}") for j in range(NMB)]
    for j in range(NMB):
        nc.tensor.matmul(p2[j][:], fc2s, x1T[:, MBs[j]], start=True, stop=True)
    for j in range(NMB):
        nc.scalar.activation(x2a[0:2, MBs[j]], p2[j][:], AF.Sigmoid, bias=fc2b)
    p3 = [fps.tile([64, MB], FP, tag="fp", name=f"p3_{j}") for j in range(NMB)]
    for j in range(NMB):
        nc.tensor.matmul(p3[j][:], fc3s, x2a[0:2, MBs[j]], start=True, stop=True)
    for j in range(NMB):
        nc.scalar.activation(c1[0:64, MBs[j]], p3[j][:], AF.Identity, bias=fc3b,
                             accum_out=xacc[:, j:j + 1])
    for j in range(NMB):
        nc.vector.tensor_copy(c2[64:128, MBs[j]], c1[0:64, MBs[j]])
    xs_f = small.tile([64, 1], FP, tag="xs_f")
    nc.vector.tensor_reduce(xs_f[:], xacc[:], AX.X, AL.add)
    nc.vector.tensor_copy(xsb[:], xs_f[:])

    # ================= GRU gates (breadth-first stages) =================
    gw = frontA.enter_context(tc.tile_pool(name="gw", bufs=4))
    zp = [fps.tile([64, MB], FP, tag="fp", name=f"zp_{j}") for j in range(NMB)]
    for j in range(NMB):
        nc.tensor.matmul(zp[j][:], wz, c1[:, MBs[j]], start=True, stop=True)
    zt = [gw.tile([64, MB], BF, tag="zt", name=f"zt_{j}") for j in range(NMB)]
    for j in range(NMB):
        nc.scalar.activation(zt[j][:], zp[j][:], AF.Sigmoid)
    rp = [fps.tile([64, MB], FP, tag="fp", name=f"rp_{j}") for j in range(NMB)]
    for j in range(NMB):
        nc.tensor.matmul(rp[j][:], wr, c1[:, MBs[j]], start=True, stop=True)
    rt = [gw.tile([64, MB], BF, tag="rt", name=f"rt_{j}") for j in range(NMB)]
    for j in range(NMB):
        nc.scalar.activation(rt[j][:], rp[j][:], AF.Sigmoid)
    for j in range(NMB):
        nc.vector.tensor_tensor(c2[0:64, MBs[j]], rt[j][:], lastT[:, MBs[j]], AL.mult)
    hp = [fps.tile([64, MB], FP, tag="fp", name=f"hp_{j}") for j in range(NMB)]
    for j in range(NMB):
        nc.tensor.matmul(hp[j][:], wh, c2[:, MBs[j]], start=True, stop=True)
    ht = [gw.tile([64, MB], BF, tag="ht", name=f"ht_{j}") for j in range(NMB)]
    for j in range(NMB):
        nc.scalar.activation(ht[j][:], hp[j][:], AF.Tanh)
    # switch the scalar act table to the sqrt set (the only switch)
    warm2 = small.tile([1, 1], FP, tag="warm2")
    nc.scalar.activation(warm2[:], fp_[0:1, F_EPS:F_EPS + 1], AF.Sqrt)
    dt_ = [gw.tile([64, MB], BF, tag="dt", name=f"dt_{j}") for j in range(NMB)]
    for j in range(NMB):
        nc.vector.tensor_tensor(dt_[j][:], ht[j][:], lastT[:, MBs[j]], AL.subtract)
    for j in range(NMB):
        nc.vector.tensor_tensor(dt_[j][:], dt_[j][:], zt[j][:], AL.mult)
    for j in range(NMB):
        nc.vector.tensor_tensor(hgsq[0:64, MBs[j]], dt_[j][:], lastT[:, MBs[j]], AL.add)
    for j in range(NMB):
        nc.vector.tensor_tensor(hgsq[64:128, MBs[j]], hgsq[0:64, MBs[j]],
                                hgsq[0:64, MBs[j]], AL.mult)

    # ---- xo squares (input-only dependent) ----
    for j in range(2):
        nc.vector.tensor_tensor(osq[64:128, HBs[j]], osq[0:64, HBs[j]],
                                osq[0:64, HBs[j]], AL.mult)

    # ============ xo per-chunk stats -> oa/oc rows ============
    oxst = fps.tile([128, 2 * HCH], FP, tag="fp", name="oxst",
                    padded_shape=[128, 512])
    for ci in range(HCH):
        csl = slice(ci * 128, (ci + 1) * 128)
        nc.tensor.matmul(oxst[:, 2 * ci:2 * ci + 2], osq[:, csl], sel2,
                         start=True, stop=True, skip_group_check=True)
    ox3 = oxst[:].rearrange("p (c s) -> p s c", s=2)
    omu = small.tile([128, HCH], FP, tag="omu")
    nc.vector.tensor_scalar(omu[:].unsqueeze(1), ox3[:, 0:1, :], 1.0 / 64,
                            None, AL.mult)
    om2 = small.tile([128, HCH], FP, tag="om2")
    nc.vector.tensor_tensor(om2[:], omu[:], omu[:], AL.mult)
    ovar = small.tile([128, HCH], FP, tag="ovar")
    nc.vector.scalar_tensor_tensor(ovar[:].unsqueeze(1), ox3[:, 1:2, :], 1.0 / 64,
                                   om2[:].unsqueeze(1), AL.mult, AL.subtract)
    osd = small.tile([128, HCH], FP, tag="osd")
    nc.scalar.activation(osd[:], ovar[:], AF.Sqrt, bias=epsc128)
    oa = small.tile([128, HCH], FP, tag="oa")
    nc.vector.reciprocal(oa[:], osd[:])
    opack = small.tile([128, 2 * HCH], BF, tag="opack")
    nc.vector.tensor_copy(opack[:, 0:HCH], oa[:])
    nc.vector.scalar_tensor_tensor(opack[:, HCH:2 * HCH], omu[:], -1.0, oa[:],
                                   AL.mult, AL.mult)
    otp = fps.tile([2 * HCH, 128], BF, tag="fp", name="otp",
                   padded_shape=[16, 1024])
    _tp(nc, otp[:], opack[:], identb)
    oT = small.tile([2 * HCH, 128], BF, tag="oT")
    nc.vector.tensor_copy(oT[:], otp[:])
    oar = small.tile([1, HALF], BF, tag="oar")
    nc.sync.dma_start(oar[:].rearrange("o (i p) -> o i p", p=128), oT[0:HCH, :])
    ocr = small.tile([1, HALF], BF, tag="ocr")
    nc.gpsimd.dma_start(ocr[:].rearrange("o (i p) -> o i p", p=128),
                        oT[HCH:2 * HCH, :])

    # ============ Hg per-chunk stats -> a_row / c row / cT_sb ============
    hst = fps.tile([128, 2 * NCH], FP, tag="fp", name="hst",
                   padded_shape=[128, 512])
    for ci in range(NCH):
        csl = slice(ci * 128, (ci + 1) * 128)
        nc.tensor.matmul(hst[:, 2 * ci:2 * ci + 2], hgsq[:, csl], sel2,
                         start=True, stop=True, skip_group_check=True)
    h3 = hst[:].rearrange("p (c s) -> p s c", s=2)
    hmu = small.tile([128, NCH], FP, tag="hmu")
    nc.vector.tensor_scalar(hmu[:].unsqueeze(1), h3[:, 0:1, :], 1.0 / 64,
                            None, AL.mult)
    hm2 = small.tile([128, NCH], FP, tag="hm2")
    nc.vector.tensor_tensor(hm2[:], hmu[:], hmu[:], AL.mult)
    hvar = small.tile([128, NCH], FP, tag="hvar")
    nc.vector.scalar_tensor_tensor(hvar[:].unsqueeze(1), h3[:, 1:2, :], 1.0 / 64,
                                   hm2[:].unsqueeze(1), AL.mult, AL.subtract)
    hsd = small.tile([128, NCH], FP, tag="hsd")
    nc.scalar.activation(hsd[:], hvar[:], AF.Sqrt, bias=epsc128)
    ha = small.tile([128, NCH], FP, tag="ha")
    nc.vector.reciprocal(ha[:], hsd[:])
    # cT_sb = -mu/sd in chunk layout (lastH bias), fp32
    nc.vector.scalar_tensor_tensor(cT_sb[:], hmu[:], -1.0, ha[:],
                                   AL.mult, AL.mult)
    hpack = small.tile([128, 2 * NCH], BF, tag="hpack")
    nc.vector.tensor_copy(hpack[:, 0:NCH], ha[:])
    nc.vector.tensor_copy(hpack[:, NCH:2 * NCH], cT_sb[:])
    htp = fps.tile([2 * NCH, 128], BF, tag="fp", name="htp",
                   padded_shape=[32, 1024])
    _tp(nc, htp[:], hpack[:], identb)
    haT = small.tile([2 * NCH, 128], BF, tag="haT")
    nc.vector.tensor_copy(haT[:], htp[:])
    nc.sync.dma_start(a_row[:].rearrange("o (i p) -> o i p", p=128), haT[0:NCH, :])
    nc.gpsimd.dma_start(HgQ[64:65, :].rearrange("o (i p) -> o i p", p=128),
                        haT[NCH:2 * NCH, :])

    # ---- xo affine into x1aug (oab/ocb broadcasts ready by now) ----
    oab = [fps.tile([64, MB], FP, tag="fp", name=f"oab_{j}") for j in range(2)]
    for j in range(2):
        nc.tensor.matmul(oab[j][:], onesr64, oar[:, HBs[j]], start=True, stop=True)
    ocb = [fps.tile([64, MB], FP, tag="fp", name=f"ocb_{j}") for j in range(2)]
    for j in range(2):
        nc.tensor.matmul(ocb[j][:], onesr64, ocr[:, HBs[j]], start=True, stop=True)
    for j in range(2):
        tb = small.tile([64, MB], BF, tag=f"oxt_{j}", name=f"oxt_{j}")
        nc.vector.tensor_tensor(tb[:], osq[0:64, HBs[j]], oab[j][:], AL.mult)
        nc.vector.tensor_tensor(tb[:], tb[:], ocb[j][:], AL.add)
        nc.scalar.activation(x1aug[0:64, HBs[j]], tb[:], AF.Identity,
                             scale=xng_c, bias=xb3_c)

    # ---- HgA = Hg_raw * a ----
    ab = [fps.tile([64, MB], FP, tag="fp", name=f"ab_{j}") for j in range(NMB)]
    for j in range(NMB):
        nc.tensor.matmul(ab[j][:], onesr64, a_row[:, MBs[j]], start=True, stop=True)
    for j in range(NMB):
        nc.vector.tensor_tensor(HgQ[0:64, MBs[j]], hgsq[0:64, MBs[j]], ab[j][:],
                                AL.mult)

    # ========== q/k projections + Gram matrices (interleaved on PE) ==========
    kacc = small.tile([128, 8], FP, tag="kacc")
    kjobs = []
    for half, dst in ((0, kT0), (1, kT1)):
        for j in range(NMB):
            kjobs.append((dst, slice(W_KA + 128 * half, W_KA + 128 * (half + 1)),
                          MBs[j], kacc[:, 4 * half + j:4 * half + j + 1]))
    qjobs = []
    for half, dst in ((0, qT0), (1, qT1)):
        for j in range(2):
            qjobs.append((dst, slice(W_QA + 128 * half, W_QA + 128 * (half + 1)),
                          HBs[j], None))
    jobs = kjobs + qjobs
    kq_ps = {}

    def leaky_evac(dst_ap, src_ap, acc, on_scalar):
        # PSUM has one DVE read port, so the vector path must evacuate
        # first and apply the leaky in place on SBUF.
        if on_scalar:
            if acc is not None:
                nc.scalar.activation(dst_ap, src_ap, AF.Prelu, alpha=0.01,
                                     accum_out=acc)
            else:
                nc.scalar.activation(dst_ap, src_ap, AF.Prelu, alpha=0.01)
        else:
            nc.vector.tensor_copy(dst_ap, src_ap)
            if acc is not None:
                nc.vector.scalar_tensor_tensor(dst_ap, dst_ap, 0.01, dst_ap,
                                               AL.mult, AL.max, accum_out=acc)
            else:
                nc.vector.scalar_tensor_tensor(dst_ap, dst_ap, 0.01, dst_ap,
                                               AL.mult, AL.max)

    gt_ps = fps.tile([128, 256], FP, tag="g", name="gt_ps", padded_shape=[128, 512], bufs=3)
    gb_ps = fps.tile([128, 256], FP, tag="g", name="gb_ps", padded_shape=[128, 512], bufs=3)
    gs_ps = fps.tile([64, 64], FP, tag="g", name="gs_ps", padded_shape=[64, 512], bufs=3)
    krp = frontA.enter_context(tc.tile_pool(name="krp", bufs=3))
    krs, xrs = {}, {}

    def gram_accum(g):
        st, sp = (g == 0), (g == NCH - 1)
        nc.tensor.matmul(gt_ps[:], krs[g][:, 0:128], krs[g][:], start=st, stop=sp)
        nc.tensor.matmul(gb_ps[:], krs[g][:, 128:256], krs[g][:], start=st, stop=sp)
        nc.tensor.matmul(gs_ps[:], xrs[g][:], xrs[g][:], start=st, stop=sp)

    for mi in range(NCH):
        msl = slice(mi * 128, (mi + 1) * 128)
        krq = fps.tile([128, 256], FP, tag="fp", name=f"krq_{mi}",
                       padded_shape=[128, 512])
        nc.tensor.matmul(krq[:], HgQ[:, msl], kA, start=True, stop=True)
        xrq = fps.tile([128, 64], FP, tag="fp", name=f"xrq_{mi}",
                       padded_shape=[128, 512])
        nc.tensor.matmul(xrq[:], x2a[:, msl], fc3a, start=True, stop=True)
        if mi < 12:
            dst, wsl, sl, acc = jobs[mi]
            kp = fps.tile([128, MB], FP, tag="fp", name=f"kqp_{mi}")
            nc.tensor.matmul(kp[:], wp[0:66, wsl], HgQ[:, sl], start=True, stop=True)
            kq_ps[mi] = kp
        kr = krp.tile([128, 256], BF, tag="kr", name=f"kr_{mi}")
        leaky_evac(kr[:], krq[:], None, mi % 2 == 0)
        xr = krp.tile([128, 64], BF, tag="xr", name=f"xr_{mi}")
        nc.vector.tensor_copy(xr[:], xrq[:])
        krs[mi], xrs[mi] = kr, xr
        if mi < 12:
            dst, wsl, sl, acc = jobs[mi]
            leaky_evac(dst[:, sl], kq_ps[mi][:], acc, mi % 2 == 1)
        if mi >= 2:
            gram_accum(mi - 2)
    gram_accum(NCH - 2)
    gram_accum(NCH - 1)
    nc.vector.tensor_copy(gt_sb[:], gt_ps[:])
    nc.vector.tensor_copy(gb_sb[:], gb_ps[:])
    nc.vector.tensor_copy(gs_f[:], gs_ps[:])
    ks_f = small.tile([128, 2], FP, tag="ks_f")
    nc.vector.tensor_reduce(ks_f[:], kacc[:].rearrange("p (h j) -> p h j", j=4),
                            AX.X, AL.add)
    nc.vector.tensor_copy(ks0[:], ks_f[:, 0:1])
    nc.vector.tensor_copy(ks1[:], ks_f[:, 1:2])

    # ga/gs broadcast tiles + k2 / x3gs
    gps_ = [fps.tile([128, MB], FP, tag="fp", name=f"gab_{j}") for j in range(NMB)]
    for j in range(NMB):
        nc.tensor.matmul(gps_[j][:], onesr128, ga_r[:, MBs[j]], start=True, stop=True)
    for j in range(NMB):
        nc.vector.tensor_copy(ga_b[:, MBs[j]], gps_[j][:])
    gss_ = [fps.tile([64, MB], FP, tag="fp", name=f"gsb_{j}") for j in range(NMB)]
    for j in range(NMB):
        nc.tensor.matmul(gss_[j][:], onesr64, gs_r[:, MBs[j]], start=True, stop=True)
    for j in range(NMB):
        nc.vector.tensor_copy(gs_b[:, MBs[j]], gss_[j][:])
    nc.vector.tensor_tensor(k2T0[:], kT0[:], ga_b[:], AL.mult)
    nc.vector.tensor_tensor(k2T1[:], kT1[:], ga_b[:], AL.mult)
    nc.vector.tensor_tensor(x3gs[0:64, :], c1[0:64, :], gs_b[:], AL.mult)

    # GCN layer-1 matmul (dl-independent)
    for jb in range(2):
        ph1 = fps.tile([64, MB], FP, tag="fp", name=f"ph1_{jb}")
        nc.tensor.matmul(ph1[:], w1a, x1aug[:, HBs[jb]], start=True, stop=True)
        nc.vector.tensor_copy(ph1sb[:, HBs[jb]], ph1[:])

    # lastH output (PE filler between kq and gram; needs HgA + cT_sb only)
    for i in range(HCH):
        pt = fps.tile([128, 64], BF, tag="fp", name=f"lpt_{i}",
                      padded_shape=[128, 1024])
        _tp(nc, pt[:], HgQ[0:64, i * 128:(i + 1) * 128], identb)
        nc.scalar.activation(lastR[:, i * 64:(i + 1) * 64], pt[:], AF.Identity,
                             bias=cT_sb[:, i:i + 1])
    l3 = lastR[:].rearrange("p (g e) -> p g e", e=64)
    lg3 = brows["bng"][:].unsqueeze(1).broadcast_to([128, HCH, 64])
    lb3 = brows["bnb"][:].unsqueeze(1).broadcast_to([128, HCH, 64])
    nc.vector.tensor_tensor(l3, l3, lg3, AL.mult)
    nc.vector.tensor_tensor(l3, l3, lb3, AL.add)

    # s1/t1 stat matmuls (only need q/c1/ks/xsb; overlap with row stats)
    sst1 = fps.tile([128, 16], FP, tag="fp", name="sst1", padded_shape=[128, 512])
    for ci in range(HCH):
        csl = slice(ci * 128, (ci + 1) * 128)
        nc.tensor.matmul(sst1[:, ci:ci + 1], qT0[:, csl], ks0[:],
                         start=True, stop=False, skip_group_check=True)
        nc.tensor.matmul(sst1[:, ci:ci + 1], qT1[:, csl], ks1[:],
                         start=False, stop=True, skip_group_check=True)
        nc.tensor.matmul(sst1[:, 8 + ci:9 + ci], c1[0:64, csl], xsb[:],
                         start=True, stop=True, skip_group_check=True)
    sstc1 = small.tile([128, 16], FP, tag="sstc1")
    nc.vector.tensor_copy(sstc1[:], sst1[:])

    frontA.close()

    # ========== own-row stats (transposed landing, no DMA scatter) ==========
    statq = ExitStack()
    ups = statq.enter_context(tc.tile_pool(name="ups", bufs=2, space="PSUM"))
    sps = statq.enter_context(tc.tile_pool(name="sps", bufs=1, space="PSUM"))
    for jb in range(2):
        sl = HBs[jb]
        ut0 = ups.tile([128, MB], FP, tag="ut", name=f"ut0_{jb}")
        nc.tensor.matmul(ut0[:], gt_sb[:, 0:128], qT0[:, sl], start=True, stop=False)
        nc.tensor.matmul(ut0[:], gb_sb[:, 0:128], qT1[:, sl], start=False, stop=True)
        ut1 = ups.tile([128, MB], FP, tag="ut", name=f"ut1_{jb}")
        nc.tensor.matmul(ut1[:], gt_sb[:, 128:256], qT0[:, sl], start=True, stop=False)
        nc.tensor.matmul(ut1[:], gb_sb[:, 128:256], qT1[:, sl], start=False, stop=True)
        nc.vector.tensor_tensor(e0sb[:, sl], ut0[:], qT0[:, sl], AL.mult)
        nc.vector.tensor_tensor(e1sb[:, sl], ut1[:], qT1[:, sl], AL.mult)
    for jb in range(2):
        sl = HBs[jb]
        us = ups.tile([64, MB], FP, tag="ut", name=f"us_{jb}")
        nc.tensor.matmul(us[:], gs_f[:], c1[0:64, sl], start=True, stop=True)
        nc.vector.tensor_tensor(essb[:, sl], us[:], c1[0:64, sl], AL.mult)

    # s2/t2 in one packed PSUM bank
    sst2 = sps.tile([128, 16], FP, tag="sst", padded_shape=[128, 512])
    for ci in range(HCH):
        csl = slice(ci * 128, (ci + 1) * 128)
        nc.tensor.matmul(sst2[:, ci:ci + 1], e0sb[:, csl], ones128c,
                         start=True, stop=False, skip_group_check=True)
        nc.tensor.matmul(sst2[:, ci:ci + 1], e1sb[:, csl], ones128c,
                         start=False, stop=True, skip_group_check=True)
        nc.tensor.matmul(sst2[:, 8 + ci:9 + ci], essb[:, csl], ones64c,
                         start=True, stop=True, skip_group_check=True)
    sstc2 = small.tile([128, 16], FP, tag="sstc2")
    nc.vector.tensor_copy(sstc2[:], sst2[:])
    statq.close()

    # ===== phase I pools (all 8 banks) =====
    zstack = ExitStack()
    zps = zstack.enter_context(tc.tile_pool(name="zps", bufs=8, space="PSUM"))
    scrv = zstack.enter_context(tc.tile_pool(name="scrv", bufs=2))
    scra = zstack.enter_context(tc.tile_pool(name="scra", bufs=2))
    ztiles = {}

    def passes12(i):
        csl = slice(i * 128, (i + 1) * 128)
        zpt = [zps.tile([128, MB], FP, tag="zpt", name=f"zp_{i}_{m}")
               for m in range(NMB)]
        ztiles[i] = zpt
        for mb in range(NMB):
            nc.tensor.matmul(zpt[mb][:], qT0[:, csl],
                             k2T0[:, mb * MB:(mb + 1) * MB],
                             start=True, stop=False)
        for mb in range(NMB):
            nc.tensor.matmul(zpt[mb][:], qT1[:, csl],
                             k2T1[:, mb * MB:(mb + 1) * MB],
                             start=False, stop=False)

    # fill the PE while the own-row stats land
    passes12(0)
    passes12(1)

    # padded sources for XBAR DMA-transposes (free dim must be 128)
    rpk = small.tile([128, 128], BF, tag="rpk")
    nc.vector.memset(rpk[:, 32:128], 0.0)
    dlpad = small.tile([128, 128], BF, tag="dlpad")
    nc.vector.memset(dlpad[:], 0.0)
    dlT = small.tile([128, 128], BF, tag="dlT")

    # ---- stat landing math (vector/scalar on [128,8] groups) ----
    smu = small.tile([128, 16], FP, tag="smu")
    nc.vector.tensor_scalar(smu[:], sstc1[:], 1.0 / N, None, AL.mult)
    sm2 = small.tile([128, 16], FP, tag="sm2")
    nc.vector.tensor_tensor(sm2[:], smu[:], smu[:], AL.mult)
    svar = small.tile([128, 16], FP, tag="svar")
    nc.vector.scalar_tensor_tensor(svar[:], sstc2[:], 1.0 / N, sm2[:],
                                   AL.mult, AL.subtract)
    ssd = small.tile([128, 16], FP, tag="ssd")
    nc.scalar.activation(ssd[:], svar[:], AF.Sqrt, bias=epsc128)
    rsS = small.tile([128, 8], FP, tag="rsS")
    nc.vector.reciprocal(rsS[:], ssd[:, 8:16])
    rho = small.tile([128, 8], FP, tag="rho")
    nc.vector.tensor_tensor(rho[:], ssd[:, 0:8], rsS[:], AL.mult)
    # rho lands first: its broadcast chain is the longest pole to pass3
    nc.vector.tensor_copy(rpk[:, 24:32], rho[:])
    nc.vector.tensor_scalar(rpk[:, 0:8], smu[:, 0:8], -1.0, None, AL.mult)
    nc.vector.scalar_tensor_tensor(rpk[:, 8:16], smu[:, 8:16], -1.0, rho[:],
                                   AL.mult, AL.mult)
    nc.vector.tensor_copy(rpk[:, 16:24], ssd[:, 0:8])
    rT = small.tile([128, 128], BF, tag="rT")
    nc.sync.dma_start(rT[:], rpk[:], transpose=True)
    rho_row = small.tile([1, HALF], BF, tag="rho_row")
    nc.gpsimd.dma_start(rho_row[:].rearrange("o (i p) -> o i p", p=128),
                        rT[24:32, :])
    x3rh = small.tile([64, HALF], BF, tag="x3rh")
    nc.gpsimd.partition_broadcast(x3rh[:], rho_row[:])
    nc.vector.tensor_tensor(x3rA[0:64, :], c1[0:64, 0:HALF], x3rh[:], AL.mult)
    for r, eng in ((0, nc.sync), (1, nc.scalar), (2, nc.sync)):
        eng.dma_start(
            x3rA[64 + r:65 + r, :].rearrange("o (i p) -> o i p", p=128),
            rT[8 * r:8 * r + 8, :])
    # lastH store, emitted late so it cannot head-of-line block the land
    nc.sync.dma_start(io["lastH"].rearrange("(i p) e -> p i e", p=128),
                      lastR[:].rearrange("p (i e) -> p i e", e=64))

    def pass3(i):
        csl = slice(i * 128, (i + 1) * 128)
        zpt = ztiles[i]
        for mb in range(NMB):
            nc.tensor.matmul(zpt[mb][:], x3rA[:, csl],
                             x3gs[:, mb * MB:(mb + 1) * MB],
                             start=False, stop=True)
        for mb in range(NMB):
            acc = rc32[:, 4 * i + mb:4 * i + mb + 1]
            if mb % 2 == 0:
                scr = scrv.tile([128, MB], BF, tag="scr", name=f"scr_{i}_{mb}")
                nc.vector.tensor_scalar(scr[:], zpt[mb][:], 0.0, None, AL.max,
                                        AL.add, accum_out=acc)
            else:
                scr = scra.tile([128, MB], BF, tag="scr2", name=f"scr2_{i}_{mb}")
                nc.scalar.activation(scr[:], zpt[mb][:], AF.Relu, accum_out=acc)
            if mb == i // 4:
                off = (i * 128) % MB
                dsel = scrv.tile([128, 128], BF, tag="dsel", name=f"dsel_{i}")
                nc.gpsimd.affine_select(
                    out=dsel[:], in_=scr[:, off:off + 128],
                    compare_op=AL.is_equal, fill=0.0, base=0,
                    pattern=[[-1, 128]], channel_multiplier=1)
                nc.vector.tensor_reduce(dg8[:, i:i + 1], dsel[:], AX.X, AL.add)

    # -------- per-pair GCN/diag/output tail, split into 3 stages --------
    # T1(p): diag/rowsum -> dls broadcast + GCN-2 input (no PE, no PSUM)
    # T2(p): GCN layers 2/3 + square (2 PSUM tiles, emitted one chunk later)
    # T3(p): final LN stats + transpose + output DMA (3 PSUM tiles)
    def tail1(p):
        c0 = 2 * p
        psl = slice(256 * p, 256 * (p + 1))
        rs2 = small.tile([128, 2], FP, tag=f"rs2_{p}", name=f"rs2_{p}")
        nc.vector.tensor_reduce(
            rs2[:], rc32[:, 8 * p:8 * p + 8].rearrange("p (i m) -> p i m", m=4),
            AX.X, AL.add)
        nc.vector.reciprocal(rs2[:], rs2[:])
        if p < 3:
            nc.vector.tensor_tensor(dlpad[:, c0:c0 + 2], dg8[:, c0:c0 + 2],
                                    rs2[:], AL.mult)
            nc.scalar.dma_start(dlT[:], dlpad[:], transpose=True)
            dl_src = dlT[c0:c0 + 2, :]
        else:
            # last pair: PSUM is free, use the short PE-transpose path
            dl2 = small.tile([128, 2], BF, tag="dl2_3", name="dl2_3")
            nc.vector.tensor_tensor(dl2[:], dg8[:, c0:c0 + 2], rs2[:], AL.mult)
            dltp = zps.tile([2, 128], BF, tag="zpt", name="dltp_3",
                            padded_shape=[2, 1024])
            _tp(nc, dltp[:], dl2[:], identb)
            dlT2 = small.tile([2, 128], BF, tag="dlT2_3", name="dlT2_3")
            nc.scalar.copy(dlT2[:], dltp[:])
            dl_src = dlT2[:]
        dlr = small.tile([1, 256], BF, tag=f"dlr_{p}", name=f"dlr_{p}")
        nc.sync.dma_start(dlr[:].rearrange("o (i p) -> o i p", p=128), dl_src)
        dls = small.tile([64, 256], BF, tag=f"dls_{p}", name=f"dls_{p}")
        nc.gpsimd.partition_broadcast(dls[:], dlr[:])
        nc.vector.tensor_tensor(hca[0:64, psl], ph1sb[:, psl], dls[:], AL.mult)
        return dls

    def tail2(p, dls):
        psl = slice(256 * p, 256 * (p + 1))
        ph2 = zps.tile([64, 256], FP, tag="zpt", name=f"ph2_{p}",
                       padded_shape=[64, 512])
        nc.tensor.matmul(ph2[:], w2a, hca[:, psl], start=True, stop=True)
        nc.vector.tensor_tensor(hcb[0:64, psl], ph2[:], dls[:], AL.mult)
        ph3 = zps.tile([64, 256], FP, tag="zpt", name=f"ph3_{p}",
                       padded_shape=[64, 512])
        nc.tensor.matmul(ph3[:], w3a, hcb[:, psl], start=True, stop=True)
        nc.vector.tensor_tensor(finsq[0:64, psl], ph3[:], dls[:], AL.mult)
        nc.vector.tensor_tensor(finsq[0:64, psl], finsq[0:64, psl],
                                x1aug[0:64, psl], AL.add)
        nc.scalar.square(finsq[64:128, psl], finsq[0:64, psl])

    def tail3(p):
        c0 = 2 * p
        fst = zps.tile([128, 4], FP, tag="zpt", name=f"fst_{p}",
                       padded_shape=[128, 512])
        for k in range(2):
            csl = slice((c0 + k) * 128, (c0 + k + 1) * 128)
            nc.tensor.matmul(fst[:, 2 * k:2 * k + 2], finsq[:, csl], sel2,
                             start=True, stop=True, skip_group_check=True)
        f3 = fst[:].rearrange("p (c s) -> p s c", s=2)
        fmu = small.tile([128, 2], FP, tag=f"fmu_{p}", name=f"fmu_{p}")
        nc.vector.tensor_scalar(fmu[:].unsqueeze(1), f3[:, 0:1, :], 1.0 / 64,
                                None, AL.mult)
        fm2 = small.tile([128, 2], FP, tag=f"fm2_{p}", name=f"fm2_{p}")
        nc.vector.tensor_tensor(fm2[:], fmu[:], fmu[:], AL.mult)
        fvar = small.tile([128, 2], FP, tag=f"fvar_{p}", name=f"fvar_{p}")
        nc.vector.scalar_tensor_tensor(fvar[:].unsqueeze(1), f3[:, 1:2, :],
                                       1.0 / 64, fm2[:].unsqueeze(1),
                                       AL.mult, AL.subtract)
        fsd = small.tile([128, 2], FP, tag=f"fsd_{p}", name=f"fsd_{p}")
        nc.scalar.activation(fsd[:], fvar[:], AF.Sqrt, bias=epsc128)
        fa = small.tile([128, 2], FP, tag=f"fa_{p}", name=f"fa_{p}")
        nc.vector.reciprocal(fa[:], fsd[:])
        fc = small.tile([128, 2], FP, tag=f"fc_{p}", name=f"fc_{p}")
        nc.vector.scalar_tensor_tensor(fc[:], fmu[:], -1.0, fa[:],
                                       AL.mult, AL.mult)
        for k in range(2):
            i = c0 + k
            ftp = zps.tile([128, 64], BF, tag="zpt", name=f"ftp_{i}",
                           padded_shape=[128, 1024])
            _tp(nc, ftp[:], finsq[0:64, i * 128:(i + 1) * 128], identb)
            nc.scalar.activation(fin[:, i * 64:(i + 1) * 64], ftp[:],
                                 AF.Identity, scale=fa[:, k:k + 1],
                                 bias=fc[:, k:k + 1])
        fpair = fin[:, 128 * p:128 * (p + 1)].rearrange("p (g e) -> p g e", e=64)
        fg3 = brows["lng"][:].unsqueeze(1).broadcast_to([128, 2, 64])
        fb3 = brows["lnb"][:].unsqueeze(1).broadcast_to([128, 2, 64])
        nc.vector.tensor_tensor(fpair, fpair, fg3, AL.mult)
        nc.vector.tensor_tensor(fpair, fpair, fb3, AL.add)
        nc.sync.dma_start(
            io["outH"][256 * p:256 * (p + 1), :].rearrange(
                "(i p) e -> p i e", p=128),
            fin[:, 128 * p:128 * (p + 1)].rearrange("p (i e) -> p i e", e=64))

    dls_of = {}
    for i in range(HCH):
        pass3(i)
        if i + 2 < HCH:
            passes12(i + 2)
        if i >= 2 and i % 2 == 0:
            tail2((i - 2) // 2, dls_of[(i - 2) // 2])
        if i >= 3 and i % 2 == 1:
            tail3((i - 3) // 2)
        if i % 2 == 1:
            dls_of[i // 2] = tail1(i // 2)
    tail2(3, dls_of[3])
    tail3(3)
    zstack.close()


def _build():
    if "nc" in _CACHE:
        return _CACHE["nc"]
    nc = bacc.Bacc("TRN2", target_bir_lowering=False, debug=False,
                   enable_asserts=True, num_devices=8)
    io = {}
    io["xT"] = nc.dram_tensor("xT", [G, N], BF, kind="ExternalInput").ap()
    io["lastT"] = nc.dram_tensor("lastT", [G, N], BF, kind="ExternalInput").ap()
    io["origT"] = nc.dram_tensor("origT", [E, HALF], BF, kind="ExternalInput").ap()
    io["corr4"] = nc.dram_tensor("corr4", [4, N], BF, kind="ExternalInput").ap()
    io["wpack"] = nc.dram_tensor("wpack", [128, WPACK_W], BF, kind="ExternalInput").ap()
    io["fpack"] = nc.dram_tensor("fpack", [128, FPACK_W], FP, kind="ExternalInput").ap()
    io["outH"] = nc.dram_tensor("outH", [HALF, E], FP, kind="ExternalOutput").ap()
    io["lastH"] = nc.dram_tensor("lastH", [HALF, G], FP, kind="ExternalOutput").ap()

    with tile.TileContext(nc) as tc:
        with ExitStack() as ctx:
            _emit(ctx, tc, io)
    nc.compile()
    nc.m = get_hw_module(nc.m)
    _CACHE["nc"] = nc
    return nc


def _host_prep(inputs):
    f32 = np.float32
    bf = ml_dtypes.bfloat16
    inp = {k: np.asarray(v, f32) for k, v in inputs.items()}
    ch = 1.0 + inp["mlp_w"].sum(axis=0)
    assert (ch > 0).all(), "head-mixing scale fold requires positive c_h"
    g, b = inp["bn_g"], inp["bn_b"]
    qw_c = inp["q_w"] * np.repeat(ch / np.sqrt(G), G)[None, :]
    Wq = g[:, None] * qw_c
    qA = np.concatenate([Wq, Wq.sum(axis=0)[None], (b @ qw_c)[None]], axis=0)
    Wk = g[:, None] * inp["k_w"]
    kA = np.concatenate([Wk, Wk.sum(axis=0)[None], (b @ inp["k_w"])[None]], axis=0)
    w1 = inp["gcn_w1"]
    w1a = np.concatenate([w1, -(inp["gcn_b3"] @ w1)[None]], axis=0)
    w2a = np.concatenate([inp["gcn_w2"], (inp["gcn_b1"] @ inp["gcn_w2"])[None]], axis=0)
    w3a = np.concatenate([inp["gcn_w3"], (inp["gcn_b2"] @ inp["gcn_w3"])[None]], axis=0)

    wpack = np.zeros((128, WPACK_W), f32)
    wpack[0:128, W_IDB:W_IDB + 128] = np.eye(128)
    wpack[0:128, W_WZ:W_WZ + 64] = inp["w_z"]
    wpack[0:128, W_WR:W_WR + 64] = inp["w_r"]
    wpack[0:128, W_WH:W_WH + 64] = inp["w_h"]
    wpack[0:66, W_QA:W_QA + 256] = qA
    wpack[0:66, W_KA:W_KA + 256] = kA
    wpack[0:64, W_FC1:W_FC1 + 16] = inp["fc1_w"]
    wpack[0:16, W_FC2:W_FC2 + 2] = inp["fc2_w"]
    wpack[0:2, W_FC3A:W_FC3A + 64] = inp["fc3_w"]
    wpack[2, W_FC3A:W_FC3A + 64] = inp["fc3_b"]
    wpack[0:65, W_W1A:W_W1A + 64] = w1a
    wpack[0:65, W_W2A:W_W2A + 64] = w2a
    wpack[0:65, W_W3A:W_W3A + 64] = w3a
    wpack[0:64, W_SEL:W_SEL + 1] = 1.0
    wpack[64:128, W_SEL + 1:W_SEL + 2] = 1.0
    wpack[:, W_ONE:W_ONE + 128] = 1.0

    fpack = np.zeros((128, FPACK_W), f32)
    fpack[0:16, F_B + 0] = inp["fc1_b"]
    fpack[0:2, F_B + 1] = inp["fc2_b"]
    fpack[0:64, F_B + 2] = inp["fc3_b"]
    fpack[0:128, F_EPS] = EPS
    fpack[0:64, F_XG] = inp["x_nom_g"]
    fpack[0:64, F_XB3] = inp["x_nom_b"] + inp["gcn_b3"]
    for k, nm in enumerate(("bn_g", "bn_b", "last_nom_g", "last_nom_b")):
        fpack[0, F_BN + 64 * k:F_BN + 64 * (k + 1)] = inp[nm]

    def c(a, dt=bf):
        return np.ascontiguousarray(np.asarray(a, dt))

    shared = {"wpack": c(wpack), "fpack": c(fpack, f32)}
    in_maps = []
    for core in range(8):
        bi, h = core // 2, core % 2
        off = h * HALF
        corr4 = np.stack([
            np.roll(inp["attn_norm_g"], -off),
            np.roll(inp["skip_norm_g"], -off),
            np.roll(inp["attn_norm_b"] + inp["skip_norm_b"], -off),
            np.ones(N, f32),
        ])
        m = dict(shared)
        m["xT"] = c(np.roll(inp["x"][bi], -off, axis=0).T)
        m["lastT"] = c(np.roll(inp["last_G_emb"][bi], -off, axis=0).T)
        m["origT"] = c(inp["orig_x"][bi, off:off + HALF].T)
        m["corr4"] = c(corr4)
        in_maps.append(m)
    return in_maps


def run(inputs, trace=False):
    nc = _build()
    in_maps = _host_prep(inputs)
    res = run_bass_kernel_spmd(nc, in_maps, core_ids=list(range(8)), trace=trace)
    out = np.zeros((B, N, E), np.float32)
    last = np.zeros((B, N, G), np.float32)
    for core in range(8):
        bi, h = core // 2, core % 2
        off = h * HALF
        out[bi, off:off + HALF] = res.results[core]["outH"]
        last[bi, off:off + HALF] = res.results[core]["lastH"]
    return (out, last), res


def kernel(**inputs):
    return run(inputs)[0]
